# revision 16
# baseline (speedup 1.0000x reference)
"""Trainium2 Bass kernel for AdvancedKANLayer.

Math (per reference):
  xn    = LayerNorm(x) * ln_w + ln_b           (eps=1e-5)
  base  = silu(xn) @ base_weight.T             [B,S,O]
  t     = tanh(xn)
  basis = cos(pi*k*t), k=1..8
  spl   = einsum('bsig,oig->bso', basis, spline_weight)
  out   = base + spl

Strategy: data-parallel over batch (8 cores, one batch entry each, no
collectives).  Per core the whole thing is one K=18432 GEMM:
  out[o, t] = sum_k W_all[k, o] * panel[k, t]
where panel rows are [silu(xn); cos(1*pi*t); ...; cos(8*pi*t)] per
I-chunk, generated on-chip.  cos(k*pi*t) is built from
c1 = cos(pi*t) = 1 - 2*sin(pi*t/2)^2 via Chebyshev product
identities on the VectorEngine (ScalarE Sin is only valid on [-pi,pi]).
Weights are pre-transposed/pre-tiled on the host, cast to bf16; matmul
runs bf16 with f32 PSUM accumulation.

K-step order is ic-major: step s = ic*9 + m (m=0 silu, m=1..8 cos_m),
so the matmul consumes panel tiles in exactly the order generation
produces them.
"""

import math

import numpy as np
import ml_dtypes

import concourse.bass as bass
import concourse.mybir as mybir
import concourse.tile as tile
from concourse import bacc
from concourse import masks
from concourse.bass import ds, ts
from concourse.bass_utils import run_bass_kernel_spmd

F32 = mybir.dt.float32
BF16 = mybir.dt.bfloat16
AF = mybir.ActivationFunctionType
ALU = mybir.AluOpType

EPS = 1e-5

# geometry (full problem, per core)
B = 8
T = 2048          # tokens per core (= S, one batch entry per core)
I = 2048          # input dim
O = 2048          # output dim
G = 8             # cos harmonics
TCH = 512         # token chunk (matmul N)
NCH = T // TCH    # 4
NIC = I // 128    # 16 I-chunks
NM = G + 1        # 9 panel row-groups per ic (silu + 8 cos)
NK = NIC * NM     # 144 k-steps of 128
KG = 8            # k-steps per weight DMA group
NG = NK // KG     # 18
NOT = O // 128    # 16 o-tiles


def build_nc(nT=T, nI=I, nO=O, tch=TCH):
    nch = nT // tch
    nic = nI // 128
    nk = nic * NM
    n_ot = nO // 128
    ntt = tch // 128          # token-tiles per chunk
    kg = KG
    while nk % kg != 0:
        kg //= 2
    ng = nk // kg

    n_race = min(5, n_ot - 1) if n_ot > 1 else 1

    nc = bacc.Bacc("TRN2", target_bir_lowering=False, debug=False)
    x_ext = nc.declare_dram_parameter("x", [nT, nI], F32, isOutput=False)
    lnw_ext = nc.declare_dram_parameter("lnw", [nI], F32, isOutput=False)
    lnb_ext = nc.declare_dram_parameter("lnb", [nI], F32, isOutput=False)
    wt_ext = nc.declare_dram_parameter("wt", [n_ot, ng, 128, kg, 128], BF16, isOutput=False)
    out_ext = nc.declare_dram_parameter("out", [nO, nT], F32, isOutput=True)

    with tile.TileContext(nc) as tc:
        with (
            tc.tile_pool(name="consts", bufs=1) as consts,
            tc.tile_pool(name="xp", bufs=4) as xpool,
            tc.tile_pool(name="statp", bufs=2) as statp,
            tc.tile_pool(name="genp", bufs=1) as genp,
            tc.tile_pool(name="ladp", bufs=1) as ladp,
            tc.tile_pool(name="panelp", bufs=1) as panelp,
            tc.tile_pool(name="wp", bufs=5) as wp,
            tc.tile_pool(name="stgp", bufs=2) as stgp,
            tc.tile_pool(name="tpps", bufs=2, space="PSUM") as tpps,
            tc.tile_pool(name="mmps", bufs=5, space="PSUM") as mmps,
        ):
            identity = consts.tile([128, 128], F32)
            masks.make_identity(nc, identity[:])
            lnw_sb = consts.tile([128, nic], F32)
            nc.sync.dma_start(lnw_sb[:], lnw_ext.rearrange("(f p) -> p f", p=128))
            lnb_sb = consts.tile([128, nic], F32)
            nc.sync.dma_start(lnb_sb[:], lnb_ext.rearrange("(f p) -> p f", p=128))
            eps_sb = consts.tile([128, 1], F32)
            nc.gpsimd.memset(eps_sb[:], EPS)
            zb = consts.tile([128, 128], BF16)
            nc.gpsimd.memset(zb[:], 0.0)

            # PE warmup: keep HAM busy while the first chunk's LN runs so
            # the first real matmuls start at full clock.
            wps = mmps.tile([128, tch], F32, tag="ps", name="warm_ps")
            for _ in range(200):
                nc.tensor.matmul(wps[:, 0:128], zb[:], zb[:])

            state = {}
            tpm = {}

            def preamble(c):
                """x DMA + LN stats + in-place normalize for chunk c.
                Stats/normalize are per token-tile so the first tile is
                ready after one x DMA, not four."""
                xnts = []
                for j in range(ntt):
                    xt = xpool.tile([128, nI], F32, tag="xt")
                    nc.sync.dma_start(xt[:], x_ext[ds((c * ntt + j) * 128, 128), :])
                    bn6 = statp.tile([128, 4, 6], F32, tag="bn6")
                    for q in range(4):
                        nc.vector.bn_stats(
                            bn6[:, q, :], xt[:, ds(q * (nI // 4), nI // 4)]
                        )
                    stats = statp.tile([128, 2], F32, tag="stats")
                    nc.vector.bn_aggr(stats[:], bn6[:])
                    std = statp.tile([128, 1], F32, tag="std")
                    nc.scalar.activation(
                        std[:], stats[:, 1:2], AF.Sqrt, bias=eps_sb[:]
                    )
                    istd = statp.tile([128, 1], F32, tag="istd")
                    nc.vector.reciprocal(istd[:], std[:])
                    nmi = statp.tile([128, 1], F32, tag="nmi")
                    nc.vector.scalar_tensor_tensor(
                        nmi[:], stats[:, 0:1], -1.0, istd[:], ALU.mult, ALU.mult
                    )
                    # normalize in place: xn = (x - mu) * istd
                    nc.scalar.activation(
                        xt[:], xt[:], AF.Identity, bias=nmi[:], scale=istd[:],
                    )
                    xnts.append(xt)
                state[c] = xnts

            def transpose_ic(c, ic):
                """PE-transpose I-chunk ic of chunk c into a PSUM tile."""
                xnts = state[c]
                tp = tpps.tile([128, tch], F32, tag="tp", name=f"tp_{c}_{ic}")
                for j in range(ntt):
                    nc.tensor.transpose(
                        tp[:, ts(j, 128)], xnts[j][:, ts(ic, 128)], identity[:]
                    )
                tpm[(c, ic)] = tp
                return tp

            pre_ptiles = {}

            def gen_ic(c, ic, ptiles):
                """Transpose + tanh/silu + cheb ladder for I-chunk ic of
                chunk c, filling ptiles[9*ic : 9*(ic+1)]."""
                tp = tpm.pop((c, ic), None)
                if tp is None:
                    tp = transpose_ic(c, ic)
                lw = lnw_sb[:, ic : ic + 1]
                lb = lnb_sb[:, ic : ic + 1]

                def pt(m):
                    s = ic * NM + m
                    t_ = panelp.tile(
                        [128, tch], BF16, tag=f"p{s:03d}", name=f"panel_{c}_{s:03d}"
                    )
                    ptiles[s] = t_
                    return t_

                th = genp.tile([128, tch], F32, tag="th")
                nc.scalar.activation(th[:], tp[:], AF.Tanh, bias=lb, scale=lw)

                p0 = pt(0)
                nc.scalar.activation(p0[:], tp[:], AF.Silu, bias=lb, scale=lw)
                sh = genp.tile([128, tch], F32, tag="sh")
                nc.scalar.activation(sh[:], th[:], AF.Sin, scale=math.pi / 2)

                def lad(tag):
                    return ladp.tile(
                        [128, tch], F32, tag=tag, name=f"lad_{tag}_{c}_{ic}"
                    )

                def stt(out, a, s, b):
                    nc.vector.scalar_tensor_tensor(
                        out[:], a[:], s, b[:], ALU.mult, ALU.mult
                    )

                # c1 = 1 - 2*sh^2
                u = lad("u")
                stt(u, sh, -2.0, sh)
                c1 = lad("c1")
                nc.vector.tensor_scalar_add(c1[:], u[:], 1.0)
                # squares on ScalarE to offload DVE
                sq1 = lad("sq")
                nc.scalar.square(sq1[:], c1[:])
                c2 = lad("c2")
                nc.vector.tensor_scalar(c2[:], sq1[:], 2.0, -1.0, ALU.mult, ALU.add)
                # c3 = 2*c1*c2 - c1
                u3 = lad("u")
                stt(u3, c2, 2.0, c1)
                c3 = lad("c3")
                nc.vector.tensor_sub(c3[:], u3[:], c1[:])

                sq2 = lad("sq")
                nc.scalar.square(sq2[:], c2[:])
                c4 = lad("c4")
                nc.vector.tensor_scalar(c4[:], sq2[:], 2.0, -1.0, ALU.mult, ALU.add)
                # exports for m=1..4 on the otherwise-idle GpSimd engine so
                # the ACT/DVE gen pipeline (which paces the race matmuls)
                # runs faster
                nc.gpsimd.tensor_copy(pt(1)[:], c1[:])
                nc.gpsimd.tensor_copy(pt(2)[:], c2[:])
                nc.gpsimd.tensor_copy(pt(3)[:], c3[:])
                nc.gpsimd.tensor_copy(pt(4)[:], c4[:])
                # leaves m=5..8 straight to panel (bf16)
                u5 = lad("u")
                stt(u5, c3, 2.0, c2)
                p5 = pt(5)
                nc.vector.tensor_sub(p5[:], u5[:], c1[:])
                sq3 = lad("sq")
                nc.scalar.square(sq3[:], c3[:])
                nc.vector.tensor_scalar(
                    pt(6)[:], sq3[:], 2.0, -1.0, ALU.mult, ALU.add
                )
                u7 = lad("u")
                stt(u7, c4, 2.0, c3)
                nc.vector.tensor_sub(pt(7)[:], u7[:], c1[:])
                sq4 = lad("sq")
                nc.scalar.square(sq4[:], c4[:])
                p8 = pt(8)
                nc.vector.tensor_scalar(
                    p8[:], sq4[:], 2.0, -1.0, ALU.mult, ALU.add
                )

            def gen_chunk(c):
                """Panel gen for chunk c (skipping pre-generated I-chunks).
                o-tile 0..3's matmul groups are emitted interleaved so the
                TensorE does real GEMM work (and stays HAM-warm) while the
                panel is being generated."""
                ptiles, pre_ic = pre_ptiles.pop(c, ([None] * nk, 0))
                pss = [
                    mmps.tile([128, tch], F32, tag="ps", name=f"ps{r}_{c}")
                    for r in range(n_race)
                ]
                g_next = 0

                def race_mm(g_hi):
                    nonlocal g_next
                    for g in range(g_next, g_hi):
                        for r in range(n_race):
                            wg = wp.tile([128, kg, 128], BF16, tag="wg",
                                         name=f"wg{r}_{c}_{g}")
                            nc.sync.dma_start(wg[:], wt_ext[r, g])
                            for k8 in range(kg):
                                sidx = g * kg + k8
                                nc.tensor.matmul(
                                    pss[r][:], wg[:, k8, :], ptiles[sidx][:],
                                    start=(sidx == 0), stop=(sidx == nk - 1),
                                )
                    g_next = g_hi

                for ic in range(nic):
                    if ic >= pre_ic:
                        gen_ic(c, ic, ptiles)
                    race_mm((NM * (ic + 1)) // kg)
                race_mm(ng)
                for r in range(n_race):
                    stg = stgp.tile([128, tch], F32, tag="stg",
                                    name=f"stg{r}_{c}")
                    nc.vector.tensor_copy(stg[:], pss[r][:])
                    nc.scalar.dma_start(
                        out_ext[ds(r * 128, 128), ds(c * tch, tch)], stg[:]
                    )
                return ptiles

            def mm_chunk(c, ptiles, nxt=None):
                if nxt is not None:
                    nxt_ptiles = [None] * nk
                    pre_ptiles[nxt] = (nxt_ptiles, 2)
                for ot in range(n_race, n_ot):
                    ps = mmps.tile([128, tch], F32, tag="ps")
                    for g in range(ng):
                        if nxt is not None and ot == n_ot - 1:
                            if g == ng - 4:
                                gen_ic(nxt, 0, nxt_ptiles)
                            elif g == ng - 2:
                                gen_ic(nxt, 1, nxt_ptiles)
                        wg = wp.tile([128, kg, 128], BF16, tag="wg")
                        nc.sync.dma_start(wg[:], wt_ext[ot, g])
                        for k8 in range(kg):
                            s = g * kg + k8
                            nc.tensor.matmul(
                                ps[:],
                                wg[:, k8, :],
                                ptiles[s][:],
                                start=(s == 0),
                                stop=(s == nk - 1),
                            )
                    stg = stgp.tile([128, tch], F32, tag="stg")
                    nc.vector.tensor_copy(stg[:], ps[:])
                    nc.scalar.dma_start(
                        out_ext[ds(ot * 128, 128), ds(c * tch, tch)], stg[:]
                    )

            preamble(0)
            for c in range(nch):
                ptiles = gen_chunk(c)
                if c + 1 < nch:
                    preamble(c + 1)
                mm_chunk(c, ptiles, nxt=(c + 1) if c + 1 < nch else None)

    _optimize_sems(nc)
    nc.compile()
    return nc


def _optimize_sems(nc):
    """Post-schedule IR pass: engine instructions complete in queue order, so
    a monotone per-engine counter semaphore only needs an increment at the
    positions some wait actually references.  Strip the rest and renumber the
    wait thresholds.  Also drop waits dominated by an earlier wait on the
    same engine queue.  Semaphores touched by DMA completions or any
    non-inc update are left alone."""
    ENG_FIFO = {
        mybir.EngineType.PE,
        mybir.EngineType.Activation,
        mybir.EngineType.DVE,
        mybir.EngineType.Pool,
        mybir.EngineType.SP,
    }
    f = nc.m.functions[0]
    insts = [i for bb in f.blocks for i in bb.instructions]

    upd_insts = {}   # sem id -> list of (inst, engine, value) in program order
    upd_ok = {}      # sem id -> eligible for stripping
    waited = {}      # sem id -> set of imm values referenced
    wait_bad = set()  # sems with register/non-ge waits
    for inst in insts:
        si = inst.sync_info
        if not si:
            continue
        is_dma = "DMA" in type(inst).__name__ or "Dma" in type(inst).__name__
        for u in (si.on_update or []):
            upd_insts.setdefault(u.id, []).append((inst, u))
            ok = upd_ok.get(u.id, True)
            if (is_dma or inst.engine not in ENG_FIFO
                    or u.update_mode != "sem-inc" or u.update_value != 1
                    or u.update_reg is not None):
                ok = False
            if any(e != inst.engine for (pi, pu) in upd_insts[u.id] for e in [pi.engine]):
                ok = False
            upd_ok[u.id] = ok
        for w in (si.on_wait or []):
            if w.wait_reg is not None or w.wait_mode != "sem-ge-imm":
                wait_bad.add(w.id)
            else:
                waited.setdefault(w.id, set()).add(w.wait_value)

    # monotone sems: every update is a positive immediate inc/add (wait-ge on
    # these can never be un-satisfied, so dominated waits are droppable)
    monotone = set()
    for sid, lst in upd_insts.items():
        if all(u.update_mode in ("sem-inc", "sem-add-imm")
               and u.update_reg is None and (u.update_value or 0) > 0
               for (_, u) in lst):
            monotone.add(sid)

    remap = {}  # sem id -> {old_val: new_val}
    keep_pos = {}  # sem id -> set of cumulative counts to keep
    for sid, lst in upd_insts.items():
        if not upd_ok.get(sid) or sid in wait_bad:
            continue
        total = len(lst)
        refs = sorted(v for v in waited.get(sid, ()) if 1 <= v <= total)
        if any(v > total or v < 1 for v in waited.get(sid, ())):
            continue
        if total not in refs:
            refs.append(total)  # keep the final count reachable for drains
        remap[sid] = {v: i + 1 for i, v in enumerate(refs)}
        keep_pos[sid] = set(refs)

    n_strip = n_keep = n_wdrop = 0
    counts = {sid: 0 for sid in remap}
    eng_wait_max = {}  # (engine, sem) -> max value already waited on that queue
    for inst in insts:
        si = inst.sync_info
        if not si:
            continue
        new_upd, new_wait, changed = [], [], False
        for u in (si.on_update or []):
            if u.id in remap:
                counts[u.id] += 1
                if counts[u.id] in keep_pos[u.id]:
                    new_upd.append(u)
                    n_keep += 1
                else:
                    changed = True
                    n_strip += 1
            else:
                new_upd.append(u)
        for w in (si.on_wait or []):
            v = w.wait_value
            if w.id in remap and w.wait_reg is None and w.wait_mode == "sem-ge-imm":
                v = remap[w.id][w.wait_value]
            key = (inst.engine, w.id)
            is_imm = w.wait_reg is None and w.wait_mode == "sem-ge-imm"
            if is_imm and w.id in monotone and eng_wait_max.get(key, 0) >= v:
                changed = True
                n_wdrop += 1
                continue
            if is_imm and w.id in monotone:
                eng_wait_max[key] = max(eng_wait_max.get(key, 0), v)
            if v != w.wait_value:
                w = mybir.SyncWait(sync_type=w.sync_type, id=w.id,
                                   ant_name=w.ant_name, wait_mode=w.wait_mode,
                                   wait_value=v, wait_reg=None)
                changed = True
            new_wait.append(w)
        if changed:
            inst.sync_info = mybir.SyncInfo(on_wait=new_wait, on_update=new_upd)
    print(f"_optimize_sems: stripped {n_strip} incs (kept {n_keep}), "
          f"dropped {n_wdrop} dominated waits")


def prep_weights(base_weight, spline_weight, nO=O, nI=I):
    """Host-side: build bf16 W_all in ic-major k-step order, pre-tiled
    for contiguous [128, kg, 128] DMAs: wt[ot, grp, k_in, ks, o_in]."""
    nic = nI // 128
    nk = nic * NM
    n_ot = nO // 128
    kg = KG
    while nk % kg != 0:
        kg //= 2
    ng = nk // kg
    w = np.empty((NM, nI, nO), np.float32)
    w[0] = base_weight.T                      # [i, o]
    for g in range(G):
        w[1 + g] = spline_weight[:, :, g].T   # [i, o]
    # m-major [9, nic, 128, nO] -> ic-major [nic, 9, 128, nO] -> [nk*128, nO]
    w = w.reshape(NM, nic, 128, nO).transpose(1, 0, 2, 3).reshape(nk * 128, nO)
    w = w.reshape(ng, kg, 128, n_ot, 128).transpose(3, 0, 2, 1, 4)
    return np.ascontiguousarray(w.astype(ml_dtypes.bfloat16))


_NC_CACHE = {}


def _get_nc():
    if "nc" not in _NC_CACHE:
        _NC_CACHE["nc"] = build_nc()
    return _NC_CACHE["nc"]


def kernel(x, ln_weight, ln_bias, base_weight, spline_weight):
    x = np.asarray(x, np.float32)
    ln_weight = np.asarray(ln_weight, np.float32)
    ln_bias = np.asarray(ln_bias, np.float32)
    wt = prep_weights(np.asarray(base_weight, np.float32),
                      np.asarray(spline_weight, np.float32))
    nc = _get_nc()
    in_maps = [
        {
            "x": np.ascontiguousarray(x[b]),
            "lnw": ln_weight,
            "lnb": ln_bias,
            "wt": wt,
        }
        for b in range(B)
    ]
    res = run_bass_kernel_spmd(nc, in_maps, core_ids=list(range(B)))
    out = np.stack([res.results[b]["out"].T for b in range(B)])
    return np.ascontiguousarray(out.astype(np.float32))



# revision 19
# speedup vs baseline: 1.1297x; 1.1297x over previous
"""Trainium2 Bass kernel for AdvancedKANLayer.

Math (per reference):
  xn    = LayerNorm(x) * ln_w + ln_b           (eps=1e-5)
  base  = silu(xn) @ base_weight.T             [B,S,O]
  t     = tanh(xn)
  basis = cos(pi*k*t), k=1..8
  spl   = einsum('bsig,oig->bso', basis, spline_weight)
  out   = base + spl

Strategy: data-parallel over batch (8 cores, one batch entry each, no
collectives).  Per core the whole thing is one K=18432 GEMM:
  out[o, t] = sum_k W_all[k, o] * panel[k, t]
where panel rows are [silu(xn); cos(1*pi*t); ...; cos(8*pi*t)] per
I-chunk, generated on-chip.  cos(k*pi*t) is built from
c1 = cos(pi*t) = 1 - 2*sin(pi*t/2)^2 via Chebyshev product
identities on the VectorEngine (ScalarE Sin is only valid on [-pi,pi]).
Weights are pre-transposed/pre-tiled on the host, cast to bf16; matmul
runs bf16 with f32 PSUM accumulation.

K-step order is ic-major: step s = ic*9 + m (m=0 silu, m=1..8 cos_m),
so the matmul consumes panel tiles in exactly the order generation
produces them.

Perf notes (measured on HW): the bf16 N=512 matmul stream floor is
~216 ns/MM and LDWEIGHTS hides fully as long as the weight DMAs stay
ahead.  To that end: output DMAs issue on the ACT HWDGE queue so the
SP queue only carries weight/x DMAs; weight DMAs move 8 k-steps per
transfer (kg=8) with a 5-deep pool; 5 o-tiles race the panel
generation (matching panel production rate ~8.7us/ic); the next
chunk's first two I-chunks are generated inside the tail of the
current mm sweep so the PE crosses chunk boundaries without idling;
200 warmup matmuls keep the HAM clock-gate open during the initial
LayerNorm; and a post-schedule pass (_optimize_sems) strips
unreferenced semaphore increments.
"""

import math
import sys
import types

try:  # some images lack antenv.axon_hooks, which bass_utils imports
    import antenv.axon_hooks  # noqa: F401
except Exception:
    try:
        import antenv
        _hooks = {}
        _m = types.ModuleType("antenv.axon_hooks")
        _m.set_axon_ntff_profile_hook = lambda h: _hooks.__setitem__("h", h)
        _m.get_axon_ntff_profile_hook = lambda: _hooks.get("h")
        sys.modules["antenv.axon_hooks"] = _m
        antenv.axon_hooks = _m
    except Exception:
        pass

import numpy as np
import ml_dtypes

import concourse.bass as bass
import concourse.mybir as mybir
import concourse.tile as tile
from concourse import bacc
from concourse import masks
from concourse.bass import ds, ts
from concourse.bass_utils import run_bass_kernel_spmd

F32 = mybir.dt.float32
BF16 = mybir.dt.bfloat16
AF = mybir.ActivationFunctionType
ALU = mybir.AluOpType

EPS = 1e-5

# geometry (full problem, per core)
B = 8
T = 2048          # tokens per core (= S, one batch entry per core)
I = 2048          # input dim
O = 2048          # output dim
G = 8             # cos harmonics
TCH = 512         # token chunk (matmul N)
NCH = T // TCH    # 4
NIC = I // 128    # 16 I-chunks
NM = G + 1        # 9 panel row-groups per ic (silu + 8 cos)
NK = NIC * NM     # 144 k-steps of 128
KG = 8            # k-steps per weight DMA group
NG = NK // KG     # 18
NOT = O // 128    # 16 o-tiles


def build_nc(nT=T, nI=I, nO=O, tch=TCH):
    nch = nT // tch
    nic = nI // 128
    nk = nic * NM
    n_ot = nO // 128
    ntt = tch // 128          # token-tiles per chunk
    kg = KG
    while nk % kg != 0:
        kg //= 2
    ng = nk // kg

    n_race = min(5, n_ot - 1) if n_ot > 1 else 1

    nc = bacc.Bacc("TRN2", target_bir_lowering=False, debug=False)
    x_ext = nc.declare_dram_parameter("x", [nT, nI], F32, isOutput=False)
    lnw_ext = nc.declare_dram_parameter("lnw", [nI], F32, isOutput=False)
    lnb_ext = nc.declare_dram_parameter("lnb", [nI], F32, isOutput=False)
    wt_ext = nc.declare_dram_parameter("wt", [n_ot, ng, 128, kg, 128], BF16, isOutput=False)
    out_ext = nc.declare_dram_parameter("out", [nO, nT], F32, isOutput=True)

    with tile.TileContext(nc) as tc:
        with (
            tc.tile_pool(name="consts", bufs=1) as consts,
            tc.tile_pool(name="xp", bufs=4) as xpool,
            tc.tile_pool(name="statp", bufs=2) as statp,
            tc.tile_pool(name="genp", bufs=1) as genp,
            tc.tile_pool(name="ladp", bufs=1) as ladp,
            tc.tile_pool(name="panelp", bufs=1) as panelp,
            tc.tile_pool(name="wp", bufs=5) as wp,
            tc.tile_pool(name="stgp", bufs=2) as stgp,
            tc.tile_pool(name="tpps", bufs=2, space="PSUM") as tpps,
            tc.tile_pool(name="mmps", bufs=5, space="PSUM") as mmps,
        ):
            identity = consts.tile([128, 128], F32)
            masks.make_identity(nc, identity[:])
            lnw_sb = consts.tile([128, nic], F32)
            nc.sync.dma_start(lnw_sb[:], lnw_ext.rearrange("(f p) -> p f", p=128))
            lnb_sb = consts.tile([128, nic], F32)
            nc.sync.dma_start(lnb_sb[:], lnb_ext.rearrange("(f p) -> p f", p=128))
            eps_sb = consts.tile([128, 1], F32)
            nc.gpsimd.memset(eps_sb[:], EPS)
            zb = consts.tile([128, 128], BF16)
            nc.gpsimd.memset(zb[:], 0.0)

            # PE warmup: keep HAM busy while the first chunk's LN runs so
            # the first real matmuls start at full clock.
            wps = mmps.tile([128, tch], F32, tag="ps", name="warm_ps")
            for _ in range(200):
                nc.tensor.matmul(wps[:, 0:128], zb[:], zb[:])

            state = {}
            tpm = {}

            def preamble(c):
                """x DMA + LN stats + in-place normalize for chunk c.
                Stats/normalize are per token-tile so the first tile is
                ready after one x DMA, not four."""
                xnts = []
                for j in range(ntt):
                    xt = xpool.tile([128, nI], F32, tag="xt")
                    nc.sync.dma_start(xt[:], x_ext[ds((c * ntt + j) * 128, 128), :])
                    bn6 = statp.tile([128, 4, 6], F32, tag="bn6")
                    for q in range(4):
                        nc.vector.bn_stats(
                            bn6[:, q, :], xt[:, ds(q * (nI // 4), nI // 4)]
                        )
                    stats = statp.tile([128, 2], F32, tag="stats")
                    nc.vector.bn_aggr(stats[:], bn6[:])
                    std = statp.tile([128, 1], F32, tag="std")
                    nc.scalar.activation(
                        std[:], stats[:, 1:2], AF.Sqrt, bias=eps_sb[:]
                    )
                    istd = statp.tile([128, 1], F32, tag="istd")
                    nc.vector.reciprocal(istd[:], std[:])
                    nmi = statp.tile([128, 1], F32, tag="nmi")
                    nc.vector.scalar_tensor_tensor(
                        nmi[:], stats[:, 0:1], -1.0, istd[:], ALU.mult, ALU.mult
                    )
                    # normalize in place: xn = (x - mu) * istd
                    nc.scalar.activation(
                        xt[:], xt[:], AF.Identity, bias=nmi[:], scale=istd[:],
                    )
                    xnts.append(xt)
                state[c] = xnts

            def transpose_ic(c, ic):
                """PE-transpose I-chunk ic of chunk c into a PSUM tile."""
                xnts = state[c]
                tp = tpps.tile([128, tch], F32, tag="tp", name=f"tp_{c}_{ic}")
                for j in range(ntt):
                    nc.tensor.transpose(
                        tp[:, ts(j, 128)], xnts[j][:, ts(ic, 128)], identity[:]
                    )
                tpm[(c, ic)] = tp
                return tp

            pre_ptiles = {}

            def gen_ic(c, ic, ptiles):
                """Transpose + tanh/silu + cheb ladder for I-chunk ic of
                chunk c, filling ptiles[9*ic : 9*(ic+1)]."""
                tp = tpm.pop((c, ic), None)
                if tp is None:
                    tp = transpose_ic(c, ic)
                lw = lnw_sb[:, ic : ic + 1]
                lb = lnb_sb[:, ic : ic + 1]

                def pt(m):
                    s = ic * NM + m
                    t_ = panelp.tile(
                        [128, tch], BF16, tag=f"p{s:03d}", name=f"panel_{c}_{s:03d}"
                    )
                    ptiles[s] = t_
                    return t_

                th = genp.tile([128, tch], F32, tag="th")
                nc.scalar.activation(th[:], tp[:], AF.Tanh, bias=lb, scale=lw)

                p0 = pt(0)
                nc.scalar.activation(p0[:], tp[:], AF.Silu, bias=lb, scale=lw)
                sh = genp.tile([128, tch], F32, tag="sh")
                nc.scalar.activation(sh[:], th[:], AF.Sin, scale=math.pi / 2)

                def lad(tag):
                    return ladp.tile(
                        [128, tch], F32, tag=tag, name=f"lad_{tag}_{c}_{ic}"
                    )

                def stt(out, a, s, b):
                    nc.vector.scalar_tensor_tensor(
                        out[:], a[:], s, b[:], ALU.mult, ALU.mult
                    )

                # c1 = 1 - 2*sh^2
                u = lad("u")
                stt(u, sh, -2.0, sh)
                c1 = lad("c1")
                nc.vector.tensor_scalar_add(c1[:], u[:], 1.0)
                # squares on ScalarE to offload DVE
                sq1 = lad("sq")
                nc.scalar.square(sq1[:], c1[:])
                c2 = lad("c2")
                nc.vector.tensor_scalar(c2[:], sq1[:], 2.0, -1.0, ALU.mult, ALU.add)
                # c3 = 2*c1*c2 - c1
                u3 = lad("u")
                stt(u3, c2, 2.0, c1)
                c3 = lad("c3")
                nc.vector.tensor_sub(c3[:], u3[:], c1[:])

                sq2 = lad("sq")
                nc.scalar.square(sq2[:], c2[:])
                c4 = lad("c4")
                nc.vector.tensor_scalar(c4[:], sq2[:], 2.0, -1.0, ALU.mult, ALU.add)
                # exports for m=1..4
                nc.scalar.copy(pt(1)[:], c1[:])
                nc.scalar.copy(pt(2)[:], c2[:])
                nc.scalar.copy(pt(3)[:], c3[:])
                nc.vector.tensor_copy(pt(4)[:], c4[:])
                # leaves m=5..8 straight to panel (bf16)
                u5 = lad("u")
                stt(u5, c3, 2.0, c2)
                p5 = pt(5)
                nc.vector.tensor_sub(p5[:], u5[:], c1[:])
                sq3 = lad("sq")
                nc.scalar.square(sq3[:], c3[:])
                nc.vector.tensor_scalar(
                    pt(6)[:], sq3[:], 2.0, -1.0, ALU.mult, ALU.add
                )
                u7 = lad("u")
                stt(u7, c4, 2.0, c3)
                nc.vector.tensor_sub(pt(7)[:], u7[:], c1[:])
                sq4 = lad("sq")
                nc.scalar.square(sq4[:], c4[:])
                p8 = pt(8)
                nc.vector.tensor_scalar(
                    p8[:], sq4[:], 2.0, -1.0, ALU.mult, ALU.add
                )

            def gen_chunk(c):
                """Panel gen for chunk c (skipping pre-generated I-chunks).
                o-tile 0..3's matmul groups are emitted interleaved so the
                TensorE does real GEMM work (and stays HAM-warm) while the
                panel is being generated."""
                ptiles, pre_ic = pre_ptiles.pop(c, ([None] * nk, 0))
                pss = [
                    mmps.tile([128, tch], F32, tag="ps", name=f"ps{r}_{c}")
                    for r in range(n_race)
                ]
                g_next = 0

                def race_mm(g_hi):
                    nonlocal g_next
                    for g in range(g_next, g_hi):
                        for r in range(n_race):
                            wg = wp.tile([128, kg, 128], BF16, tag="wg",
                                         name=f"wg{r}_{c}_{g}")
                            nc.sync.dma_start(wg[:], wt_ext[r, g])
                            for k8 in range(kg):
                                sidx = g * kg + k8
                                nc.tensor.matmul(
                                    pss[r][:], wg[:, k8, :], ptiles[sidx][:],
                                    start=(sidx == 0), stop=(sidx == nk - 1),
                                )
                    g_next = g_hi

                for ic in range(nic):
                    if ic >= pre_ic:
                        gen_ic(c, ic, ptiles)
                    race_mm((NM * (ic + 1)) // kg)
                race_mm(ng)
                for r in range(n_race):
                    stg = stgp.tile([128, tch], F32, tag="stg",
                                    name=f"stg{r}_{c}")
                    nc.vector.tensor_copy(stg[:], pss[r][:])
                    nc.scalar.dma_start(
                        out_ext[ds(r * 128, 128), ds(c * tch, tch)], stg[:]
                    )
                return ptiles

            def mm_chunk(c, ptiles, nxt=None):
                if nxt is not None:
                    nxt_ptiles = [None] * nk
                    pre_ptiles[nxt] = (nxt_ptiles, 2)
                for ot in range(n_race, n_ot):
                    ps = mmps.tile([128, tch], F32, tag="ps")
                    for g in range(ng):
                        if nxt is not None and ot == n_ot - 1:
                            if g == ng - 4:
                                gen_ic(nxt, 0, nxt_ptiles)
                            elif g == ng - 2:
                                gen_ic(nxt, 1, nxt_ptiles)
                        wg = wp.tile([128, kg, 128], BF16, tag="wg")
                        nc.sync.dma_start(wg[:], wt_ext[ot, g])
                        for k8 in range(kg):
                            s = g * kg + k8
                            nc.tensor.matmul(
                                ps[:],
                                wg[:, k8, :],
                                ptiles[s][:],
                                start=(s == 0),
                                stop=(s == nk - 1),
                            )
                    stg = stgp.tile([128, tch], F32, tag="stg")
                    nc.vector.tensor_copy(stg[:], ps[:])
                    nc.scalar.dma_start(
                        out_ext[ds(ot * 128, 128), ds(c * tch, tch)], stg[:]
                    )

            preamble(0)
            for c in range(nch):
                ptiles = gen_chunk(c)
                if c + 1 < nch:
                    preamble(c + 1)
                mm_chunk(c, ptiles, nxt=(c + 1) if c + 1 < nch else None)

    _optimize_sems(nc)
    nc.compile()
    return nc


def _optimize_sems(nc):
    """Post-schedule IR pass: engine instructions complete in queue order, so
    a monotone per-engine counter semaphore only needs an increment at the
    positions some wait actually references.  Strip the rest and renumber the
    wait thresholds.  Also drop waits dominated by an earlier wait on the
    same engine queue.  Semaphores touched by DMA completions or any
    non-inc update are left alone."""
    ENG_FIFO = {
        mybir.EngineType.PE,
        mybir.EngineType.Activation,
        mybir.EngineType.DVE,
        mybir.EngineType.Pool,
        mybir.EngineType.SP,
    }
    f = nc.m.functions[0]
    insts = [i for bb in f.blocks for i in bb.instructions]

    upd_insts = {}   # sem id -> list of (inst, engine, value) in program order
    upd_ok = {}      # sem id -> eligible for stripping
    waited = {}      # sem id -> set of imm values referenced
    wait_bad = set()  # sems with register/non-ge waits
    for inst in insts:
        si = inst.sync_info
        if not si:
            continue
        is_dma = "DMA" in type(inst).__name__ or "Dma" in type(inst).__name__
        for u in (si.on_update or []):
            upd_insts.setdefault(u.id, []).append((inst, u))
            ok = upd_ok.get(u.id, True)
            if (is_dma or inst.engine not in ENG_FIFO
                    or u.update_mode != "sem-inc" or u.update_value != 1
                    or u.update_reg is not None):
                ok = False
            if any(e != inst.engine for (pi, pu) in upd_insts[u.id] for e in [pi.engine]):
                ok = False
            upd_ok[u.id] = ok
        for w in (si.on_wait or []):
            if w.wait_reg is not None or w.wait_mode != "sem-ge-imm":
                wait_bad.add(w.id)
            else:
                waited.setdefault(w.id, set()).add(w.wait_value)

    # monotone sems: every update is a positive immediate inc/add (wait-ge on
    # these can never be un-satisfied, so dominated waits are droppable)
    monotone = set()
    for sid, lst in upd_insts.items():
        if all(u.update_mode in ("sem-inc", "sem-add-imm")
               and u.update_reg is None and (u.update_value or 0) > 0
               for (_, u) in lst):
            monotone.add(sid)

    remap = {}  # sem id -> {old_val: new_val}
    keep_pos = {}  # sem id -> set of cumulative counts to keep
    for sid, lst in upd_insts.items():
        if not upd_ok.get(sid) or sid in wait_bad:
            continue
        total = len(lst)
        refs = sorted(v for v in waited.get(sid, ()) if 1 <= v <= total)
        if any(v > total or v < 1 for v in waited.get(sid, ())):
            continue
        if total not in refs:
            refs.append(total)  # keep the final count reachable for drains
        remap[sid] = {v: i + 1 for i, v in enumerate(refs)}
        keep_pos[sid] = set(refs)

    n_strip = n_keep = n_wdrop = 0
    counts = {sid: 0 for sid in remap}
    eng_wait_max = {}  # (engine, sem) -> max value already waited on that queue
    for inst in insts:
        si = inst.sync_info
        if not si:
            continue
        new_upd, new_wait, changed = [], [], False
        for u in (si.on_update or []):
            if u.id in remap:
                counts[u.id] += 1
                if counts[u.id] in keep_pos[u.id]:
                    new_upd.append(u)
                    n_keep += 1
                else:
                    changed = True
                    n_strip += 1
            else:
                new_upd.append(u)
        for w in (si.on_wait or []):
            v = w.wait_value
            if w.id in remap and w.wait_reg is None and w.wait_mode == "sem-ge-imm":
                v = remap[w.id][w.wait_value]
            key = (inst.engine, w.id)
            is_imm = w.wait_reg is None and w.wait_mode == "sem-ge-imm"
            if is_imm and w.id in monotone and eng_wait_max.get(key, 0) >= v:
                changed = True
                n_wdrop += 1
                continue
            if is_imm and w.id in monotone:
                eng_wait_max[key] = max(eng_wait_max.get(key, 0), v)
            if v != w.wait_value:
                w = mybir.SyncWait(sync_type=w.sync_type, id=w.id,
                                   ant_name=w.ant_name, wait_mode=w.wait_mode,
                                   wait_value=v, wait_reg=None)
                changed = True
            new_wait.append(w)
        if changed:
            inst.sync_info = mybir.SyncInfo(on_wait=new_wait, on_update=new_upd)
    print(f"_optimize_sems: stripped {n_strip} incs (kept {n_keep}), "
          f"dropped {n_wdrop} dominated waits")


def prep_weights(base_weight, spline_weight, nO=O, nI=I):
    """Host-side: build bf16 W_all in ic-major k-step order, pre-tiled
    for contiguous [128, kg, 128] DMAs: wt[ot, grp, k_in, ks, o_in]."""
    nic = nI // 128
    nk = nic * NM
    n_ot = nO // 128
    kg = KG
    while nk % kg != 0:
        kg //= 2
    ng = nk // kg
    w = np.empty((NM, nI, nO), np.float32)
    w[0] = base_weight.T                      # [i, o]
    for g in range(G):
        w[1 + g] = spline_weight[:, :, g].T   # [i, o]
    # m-major [9, nic, 128, nO] -> ic-major [nic, 9, 128, nO] -> [nk*128, nO]
    w = w.reshape(NM, nic, 128, nO).transpose(1, 0, 2, 3).reshape(nk * 128, nO)
    w = w.reshape(ng, kg, 128, n_ot, 128).transpose(3, 0, 2, 1, 4)
    return np.ascontiguousarray(w.astype(ml_dtypes.bfloat16))


_NC_CACHE = {}


def _get_nc():
    if "nc" not in _NC_CACHE:
        _NC_CACHE["nc"] = build_nc()
    return _NC_CACHE["nc"]


def kernel(x, ln_weight, ln_bias, base_weight, spline_weight):
    x = np.asarray(x, np.float32)
    ln_weight = np.asarray(ln_weight, np.float32)
    ln_bias = np.asarray(ln_bias, np.float32)
    wt = prep_weights(np.asarray(base_weight, np.float32),
                      np.asarray(spline_weight, np.float32))
    nc = _get_nc()
    in_maps = [
        {
            "x": np.ascontiguousarray(x[b]),
            "lnw": ln_weight,
            "lnb": ln_bias,
            "wt": wt,
        }
        for b in range(B)
    ]
    res = run_bass_kernel_spmd(nc, in_maps, core_ids=list(range(B)))
    out = np.stack([res.results[b]["out"].T for b in range(B)])
    return np.ascontiguousarray(out.astype(np.float32))



# revision 20
# speedup vs baseline: 1.1356x; 1.0052x over previous
"""Trainium2 Bass kernel for AdvancedKANLayer.

Math (per reference):
  xn    = LayerNorm(x) * ln_w + ln_b           (eps=1e-5)
  base  = silu(xn) @ base_weight.T             [B,S,O]
  t     = tanh(xn)
  basis = cos(pi*k*t), k=1..8
  spl   = einsum('bsig,oig->bso', basis, spline_weight)
  out   = base + spl

Strategy: data-parallel over batch (8 cores, one batch entry each, no
collectives).  Per core the whole thing is one K=18432 GEMM:
  out[o, t] = sum_k W_all[k, o] * panel[k, t]
where panel rows are [silu(xn); cos(1*pi*t); ...; cos(8*pi*t)] per
I-chunk, generated on-chip.  cos(k*pi*t) is built from
c1 = cos(pi*t) = 1 - 2*sin(pi*t/2)^2 via Chebyshev product
identities on the VectorEngine (ScalarE Sin is only valid on [-pi,pi]).
Weights are pre-transposed/pre-tiled on the host, cast to bf16; matmul
runs bf16 with f32 PSUM accumulation.

K-step order is ic-major: step s = ic*9 + m (m=0 silu, m=1..8 cos_m),
so the matmul consumes panel tiles in exactly the order generation
produces them.

Perf notes (measured on HW): the bf16 N=512 matmul stream floor is
~216 ns/MM and LDWEIGHTS hides fully as long as the weight DMAs stay
ahead.  To that end: output DMAs issue on the ACT HWDGE queue so the
SP queue only carries weight/x DMAs; weight DMAs move 8 k-steps per
transfer (kg=8) with a 5-deep pool; 5 o-tiles race the panel
generation (matching panel production rate ~8.7us/ic); the next
chunk's first two I-chunks are generated inside the tail of the
current mm sweep so the PE crosses chunk boundaries without idling;
200 warmup matmuls keep the HAM clock-gate open during the initial
LayerNorm; and a post-schedule pass (_optimize_sems) strips
unreferenced semaphore increments.
"""

import math
import sys
import types

try:  # some images lack antenv.axon_hooks, which bass_utils imports
    import antenv.axon_hooks  # noqa: F401
except Exception:
    try:
        import antenv
        _hooks = {}
        _m = types.ModuleType("antenv.axon_hooks")
        _m.set_axon_ntff_profile_hook = lambda h: _hooks.__setitem__("h", h)
        _m.get_axon_ntff_profile_hook = lambda: _hooks.get("h")
        sys.modules["antenv.axon_hooks"] = _m
        antenv.axon_hooks = _m
    except Exception:
        pass

import numpy as np
import ml_dtypes

import concourse.bass as bass
import concourse.mybir as mybir
import concourse.tile as tile
from concourse import bacc
from concourse import masks
from concourse.bass import ds, ts
from concourse.bass_utils import run_bass_kernel_spmd

F32 = mybir.dt.float32
BF16 = mybir.dt.bfloat16
AF = mybir.ActivationFunctionType
ALU = mybir.AluOpType

EPS = 1e-5

# geometry (full problem, per core)
B = 8
T = 2048          # tokens per core (= S, one batch entry per core)
I = 2048          # input dim
O = 2048          # output dim
G = 8             # cos harmonics
TCH = 512         # token chunk (matmul N)
NCH = T // TCH    # 4
NIC = I // 128    # 16 I-chunks
NM = G + 1        # 9 panel row-groups per ic (silu + 8 cos)
NK = NIC * NM     # 144 k-steps of 128
KG = 8            # k-steps per weight DMA group
NG = NK // KG     # 18
NOT = O // 128    # 16 o-tiles


def build_nc(nT=T, nI=I, nO=O, tch=TCH):
    nch = nT // tch
    nic = nI // 128
    nk = nic * NM
    n_ot = nO // 128
    ntt = tch // 128          # token-tiles per chunk
    kg = KG
    while nk % kg != 0:
        kg //= 2
    ng = nk // kg

    n_race = min(5, n_ot - 1) if n_ot > 1 else 1

    nc = bacc.Bacc("TRN2", target_bir_lowering=False, debug=False)
    x_ext = nc.declare_dram_parameter("x", [nT, nI], F32, isOutput=False)
    lnw_ext = nc.declare_dram_parameter("lnw", [nI], F32, isOutput=False)
    lnb_ext = nc.declare_dram_parameter("lnb", [nI], F32, isOutput=False)
    wt_ext = nc.declare_dram_parameter("wt", [n_ot, ng, 128, kg, 128], BF16, isOutput=False)
    out_ext = nc.declare_dram_parameter("out", [nO, nT], F32, isOutput=True)

    with tile.TileContext(nc) as tc:
        with (
            tc.tile_pool(name="consts", bufs=1) as consts,
            tc.tile_pool(name="xp", bufs=4) as xpool,
            tc.tile_pool(name="statp", bufs=2) as statp,
            tc.tile_pool(name="genp", bufs=1) as genp,
            tc.tile_pool(name="ladp", bufs=1) as ladp,
            tc.tile_pool(name="panelp", bufs=1) as panelp,
            tc.tile_pool(name="wp", bufs=5) as wp,
            tc.tile_pool(name="stgp", bufs=2) as stgp,
            tc.tile_pool(name="tpps", bufs=3, space="PSUM") as tpps,
            tc.tile_pool(name="mmps", bufs=5, space="PSUM") as mmps,
        ):
            identity = consts.tile([128, 128], F32)
            masks.make_identity(nc, identity[:])
            lnw_sb = consts.tile([128, nic], F32)
            nc.sync.dma_start(lnw_sb[:], lnw_ext.rearrange("(f p) -> p f", p=128))
            lnb_sb = consts.tile([128, nic], F32)
            nc.sync.dma_start(lnb_sb[:], lnb_ext.rearrange("(f p) -> p f", p=128))
            eps_sb = consts.tile([128, 1], F32)
            nc.vector.memset(eps_sb[:], EPS)
            zb = consts.tile([128, 128], BF16)
            nc.vector.memset(zb[:], 0.0)

            # PE warmup: keep HAM busy while the first chunk's LN runs so
            # the first real matmuls start at full clock.
            wps = mmps.tile([128, tch], F32, tag="ps", name="warm_ps")
            for _ in range(200):
                nc.tensor.matmul(wps[:, 0:128], zb[:], zb[:])

            state = {}
            tpm = {}

            def preamble(c):
                """x DMA + LN stats + in-place normalize for chunk c.
                Stats/normalize are per token-tile so the first tile is
                ready after one x DMA, not four."""
                xnts = []
                for j in range(ntt):
                    xt = xpool.tile([128, nI], F32, tag="xt")
                    nc.sync.dma_start(xt[:], x_ext[ds((c * ntt + j) * 128, 128), :])
                    bn6 = statp.tile([128, 4, 6], F32, tag="bn6")
                    for q in range(4):
                        nc.vector.bn_stats(
                            bn6[:, q, :], xt[:, ds(q * (nI // 4), nI // 4)]
                        )
                    stats = statp.tile([128, 2], F32, tag="stats")
                    nc.vector.bn_aggr(stats[:], bn6[:])
                    std = statp.tile([128, 1], F32, tag="std")
                    nc.scalar.activation(
                        std[:], stats[:, 1:2], AF.Sqrt, bias=eps_sb[:]
                    )
                    istd = statp.tile([128, 1], F32, tag="istd")
                    nc.vector.reciprocal(istd[:], std[:])
                    nmi = statp.tile([128, 1], F32, tag="nmi")
                    nc.vector.scalar_tensor_tensor(
                        nmi[:], stats[:, 0:1], -1.0, istd[:], ALU.mult, ALU.mult
                    )
                    # normalize in place: xn = (x - mu) * istd
                    nc.scalar.activation(
                        xt[:], xt[:], AF.Identity, bias=nmi[:], scale=istd[:],
                    )
                    xnts.append(xt)
                state[c] = xnts

            def transpose_ic(c, ic):
                """PE-transpose I-chunk ic of chunk c into a PSUM tile."""
                xnts = state[c]
                tp = tpps.tile([128, tch], F32, tag="tp", name=f"tp_{c}_{ic}")
                for j in range(ntt):
                    nc.tensor.transpose(
                        tp[:, ts(j, 128)], xnts[j][:, ts(ic, 128)], identity[:]
                    )
                tpm[(c, ic)] = tp
                return tp

            pre_ptiles = {}

            def gen_ic(c, ic, ptiles):
                """Transpose + tanh/silu + cheb ladder for I-chunk ic of
                chunk c, filling ptiles[9*ic : 9*(ic+1)]."""
                tp = tpm.pop((c, ic), None)
                if tp is None:
                    tp = transpose_ic(c, ic)
                lw = lnw_sb[:, ic : ic + 1]
                lb = lnb_sb[:, ic : ic + 1]

                def pt(m):
                    s = ic * NM + m
                    t_ = panelp.tile(
                        [128, tch], BF16, tag=f"p{s:03d}", name=f"panel_{c}_{s:03d}"
                    )
                    ptiles[s] = t_
                    return t_

                th = genp.tile([128, tch], F32, tag="th")
                nc.scalar.activation(th[:], tp[:], AF.Tanh, bias=lb, scale=lw)

                p0 = pt(0)
                nc.scalar.activation(p0[:], tp[:], AF.Silu, bias=lb, scale=lw)
                sh = genp.tile([128, tch], F32, tag="sh")
                nc.scalar.activation(sh[:], th[:], AF.Sin, scale=math.pi / 2)

                def lad(tag):
                    return ladp.tile(
                        [128, tch], F32, tag=tag, name=f"lad_{tag}_{c}_{ic}"
                    )

                def stt(out, a, s, b):
                    nc.vector.scalar_tensor_tensor(
                        out[:], a[:], s, b[:], ALU.mult, ALU.mult
                    )

                # c1 = 1 - 2*sh^2
                u = lad("u")
                stt(u, sh, -2.0, sh)
                c1 = lad("c1")
                nc.vector.tensor_scalar_add(c1[:], u[:], 1.0)
                # squares on ScalarE to offload DVE
                sq1 = lad("sq")
                nc.scalar.square(sq1[:], c1[:])
                c2 = lad("c2")
                nc.vector.tensor_scalar(c2[:], sq1[:], 2.0, -1.0, ALU.mult, ALU.add)
                # c3 = 2*c1*c2 - c1
                u3 = lad("u")
                stt(u3, c2, 2.0, c1)
                c3 = lad("c3")
                nc.vector.tensor_sub(c3[:], u3[:], c1[:])

                sq2 = lad("sq")
                nc.scalar.square(sq2[:], c2[:])
                c4 = lad("c4")
                nc.vector.tensor_scalar(c4[:], sq2[:], 2.0, -1.0, ALU.mult, ALU.add)
                # exports for m=1..4
                nc.scalar.copy(pt(1)[:], c1[:])
                nc.scalar.copy(pt(2)[:], c2[:])
                nc.scalar.copy(pt(3)[:], c3[:])
                nc.vector.tensor_copy(pt(4)[:], c4[:])
                # leaves m=5..8 straight to panel (bf16)
                u5 = lad("u")
                stt(u5, c3, 2.0, c2)
                p5 = pt(5)
                nc.vector.tensor_sub(p5[:], u5[:], c1[:])
                sq3 = lad("sq")
                nc.scalar.square(sq3[:], c3[:])
                nc.vector.tensor_scalar(
                    pt(6)[:], sq3[:], 2.0, -1.0, ALU.mult, ALU.add
                )
                u7 = lad("u")
                stt(u7, c4, 2.0, c3)
                nc.vector.tensor_sub(pt(7)[:], u7[:], c1[:])
                sq4 = lad("sq")
                nc.scalar.square(sq4[:], c4[:])
                p8 = pt(8)
                nc.vector.tensor_scalar(
                    p8[:], sq4[:], 2.0, -1.0, ALU.mult, ALU.add
                )

            def gen_chunk(c):
                """Panel gen for chunk c (skipping pre-generated I-chunks).
                o-tile 0..3's matmul groups are emitted interleaved so the
                TensorE does real GEMM work (and stays HAM-warm) while the
                panel is being generated."""
                ptiles, pre_ic = pre_ptiles.pop(c, ([None] * nk, 0))
                pss = [
                    mmps.tile([128, tch], F32, tag="ps", name=f"ps{r}_{c}")
                    for r in range(n_race)
                ]
                g_next = 0

                def race_mm(g_hi):
                    nonlocal g_next
                    for g in range(g_next, g_hi):
                        for r in range(n_race):
                            wg = wp.tile([128, kg, 128], BF16, tag="wg",
                                         name=f"wg{r}_{c}_{g}")
                            nc.sync.dma_start(wg[:], wt_ext[r, g])
                            for k8 in range(kg):
                                sidx = g * kg + k8
                                nc.tensor.matmul(
                                    pss[r][:], wg[:, k8, :], ptiles[sidx][:],
                                    start=(sidx == 0), stop=(sidx == nk - 1),
                                )
                    g_next = g_hi

                for ic in range(nic):
                    if ic >= pre_ic:
                        gen_ic(c, ic, ptiles)
                    race_mm((NM * (ic + 1)) // kg)
                race_mm(ng)
                for r in range(n_race):
                    stg = stgp.tile([128, tch], F32, tag="stg",
                                    name=f"stg{r}_{c}")
                    nc.vector.tensor_copy(stg[:], pss[r][:])
                    nc.scalar.dma_start(
                        out_ext[ds(r * 128, 128), ds(c * tch, tch)], stg[:]
                    )
                return ptiles

            def mm_chunk(c, ptiles, nxt=None):
                if nxt is not None:
                    nxt_ptiles = [None] * nk
                    pre_ptiles[nxt] = (nxt_ptiles, 3)
                for ot in range(n_race, n_ot):
                    ps = mmps.tile([128, tch], F32, tag="ps")
                    for g in range(ng):
                        if nxt is not None and ot == n_ot - 1:
                            if g == ng - 6:
                                gen_ic(nxt, 0, nxt_ptiles)
                            elif g == ng - 4:
                                gen_ic(nxt, 1, nxt_ptiles)
                            elif g == ng - 2:
                                gen_ic(nxt, 2, nxt_ptiles)
                        wg = wp.tile([128, kg, 128], BF16, tag="wg")
                        nc.sync.dma_start(wg[:], wt_ext[ot, g])
                        for k8 in range(kg):
                            s = g * kg + k8
                            nc.tensor.matmul(
                                ps[:],
                                wg[:, k8, :],
                                ptiles[s][:],
                                start=(s == 0),
                                stop=(s == nk - 1),
                            )
                    stg = stgp.tile([128, tch], F32, tag="stg")
                    nc.vector.tensor_copy(stg[:], ps[:])
                    nc.scalar.dma_start(
                        out_ext[ds(ot * 128, 128), ds(c * tch, tch)], stg[:]
                    )

            preamble(0)
            for c in range(nch):
                ptiles = gen_chunk(c)
                if c + 1 < nch:
                    preamble(c + 1)
                mm_chunk(c, ptiles, nxt=(c + 1) if c + 1 < nch else None)

    _optimize_sems(nc)
    nc.compile()
    return nc


def _optimize_sems(nc):
    """Post-schedule IR pass: engine instructions complete in queue order, so
    a monotone per-engine counter semaphore only needs an increment at the
    positions some wait actually references.  Strip the rest and renumber the
    wait thresholds.  Also drop waits dominated by an earlier wait on the
    same engine queue.  Semaphores touched by DMA completions or any
    non-inc update are left alone."""
    ENG_FIFO = {
        mybir.EngineType.PE,
        mybir.EngineType.Activation,
        mybir.EngineType.DVE,
        mybir.EngineType.Pool,
        mybir.EngineType.SP,
    }
    f = nc.m.functions[0]
    insts = [i for bb in f.blocks for i in bb.instructions]

    upd_insts = {}   # sem id -> list of (inst, engine, value) in program order
    upd_ok = {}      # sem id -> eligible for stripping
    waited = {}      # sem id -> set of imm values referenced
    wait_bad = set()  # sems with register/non-ge waits
    for inst in insts:
        si = inst.sync_info
        if not si:
            continue
        is_dma = "DMA" in type(inst).__name__ or "Dma" in type(inst).__name__
        for u in (si.on_update or []):
            upd_insts.setdefault(u.id, []).append((inst, u))
            ok = upd_ok.get(u.id, True)
            if (is_dma or inst.engine not in ENG_FIFO
                    or u.update_mode != "sem-inc" or u.update_value != 1
                    or u.update_reg is not None):
                ok = False
            if any(e != inst.engine for (pi, pu) in upd_insts[u.id] for e in [pi.engine]):
                ok = False
            upd_ok[u.id] = ok
        for w in (si.on_wait or []):
            if w.wait_reg is not None or w.wait_mode != "sem-ge-imm":
                wait_bad.add(w.id)
            else:
                waited.setdefault(w.id, set()).add(w.wait_value)

    # monotone sems: every update is a positive immediate inc/add (wait-ge on
    # these can never be un-satisfied, so dominated waits are droppable)
    monotone = set()
    for sid, lst in upd_insts.items():
        if all(u.update_mode in ("sem-inc", "sem-add-imm")
               and u.update_reg is None and (u.update_value or 0) > 0
               for (_, u) in lst):
            monotone.add(sid)

    remap = {}  # sem id -> {old_val: new_val}
    keep_pos = {}  # sem id -> set of cumulative counts to keep
    for sid, lst in upd_insts.items():
        if not upd_ok.get(sid) or sid in wait_bad:
            continue
        total = len(lst)
        refs = sorted(v for v in waited.get(sid, ()) if 1 <= v <= total)
        if any(v > total or v < 1 for v in waited.get(sid, ())):
            continue
        if total not in refs:
            refs.append(total)  # keep the final count reachable for drains
        remap[sid] = {v: i + 1 for i, v in enumerate(refs)}
        keep_pos[sid] = set(refs)

    n_strip = n_keep = n_wdrop = 0
    counts = {sid: 0 for sid in remap}
    eng_wait_max = {}  # (engine, sem) -> max value already waited on that queue
    for inst in insts:
        si = inst.sync_info
        if not si:
            continue
        new_upd, new_wait, changed = [], [], False
        for u in (si.on_update or []):
            if u.id in remap:
                counts[u.id] += 1
                if counts[u.id] in keep_pos[u.id]:
                    new_upd.append(u)
                    n_keep += 1
                else:
                    changed = True
                    n_strip += 1
            else:
                new_upd.append(u)
        for w in (si.on_wait or []):
            v = w.wait_value
            if w.id in remap and w.wait_reg is None and w.wait_mode == "sem-ge-imm":
                v = remap[w.id][w.wait_value]
            key = (inst.engine, w.id)
            is_imm = w.wait_reg is None and w.wait_mode == "sem-ge-imm"
            if is_imm and w.id in monotone and eng_wait_max.get(key, 0) >= v:
                changed = True
                n_wdrop += 1
                continue
            if is_imm and w.id in monotone:
                eng_wait_max[key] = max(eng_wait_max.get(key, 0), v)
            if v != w.wait_value:
                w = mybir.SyncWait(sync_type=w.sync_type, id=w.id,
                                   ant_name=w.ant_name, wait_mode=w.wait_mode,
                                   wait_value=v, wait_reg=None)
                changed = True
            new_wait.append(w)
        if changed:
            inst.sync_info = mybir.SyncInfo(on_wait=new_wait, on_update=new_upd)
    print(f"_optimize_sems: stripped {n_strip} incs (kept {n_keep}), "
          f"dropped {n_wdrop} dominated waits")


def prep_weights(base_weight, spline_weight, nO=O, nI=I):
    """Host-side: build bf16 W_all in ic-major k-step order, pre-tiled
    for contiguous [128, kg, 128] DMAs: wt[ot, grp, k_in, ks, o_in]."""
    nic = nI // 128
    nk = nic * NM
    n_ot = nO // 128
    kg = KG
    while nk % kg != 0:
        kg //= 2
    ng = nk // kg
    w = np.empty((NM, nI, nO), np.float32)
    w[0] = base_weight.T                      # [i, o]
    for g in range(G):
        w[1 + g] = spline_weight[:, :, g].T   # [i, o]
    # m-major [9, nic, 128, nO] -> ic-major [nic, 9, 128, nO] -> [nk*128, nO]
    w = w.reshape(NM, nic, 128, nO).transpose(1, 0, 2, 3).reshape(nk * 128, nO)
    w = w.reshape(ng, kg, 128, n_ot, 128).transpose(3, 0, 2, 1, 4)
    return np.ascontiguousarray(w.astype(ml_dtypes.bfloat16))


_NC_CACHE = {}


def _get_nc():
    if "nc" not in _NC_CACHE:
        _NC_CACHE["nc"] = build_nc()
    return _NC_CACHE["nc"]


def kernel(x, ln_weight, ln_bias, base_weight, spline_weight):
    x = np.asarray(x, np.float32)
    ln_weight = np.asarray(ln_weight, np.float32)
    ln_bias = np.asarray(ln_bias, np.float32)
    wt = prep_weights(np.asarray(base_weight, np.float32),
                      np.asarray(spline_weight, np.float32))
    nc = _get_nc()
    in_maps = [
        {
            "x": np.ascontiguousarray(x[b]),
            "lnw": ln_weight,
            "lnb": ln_bias,
            "wt": wt,
        }
        for b in range(B)
    ]
    res = run_bass_kernel_spmd(nc, in_maps, core_ids=list(range(B)))
    out = np.stack([res.results[b]["out"].T for b in range(B)])
    return np.ascontiguousarray(out.astype(np.float32))



# revision 21
# speedup vs baseline: 1.1413x; 1.0050x over previous
"""Trainium2 Bass kernel for AdvancedKANLayer.

Math (per reference):
  xn    = LayerNorm(x) * ln_w + ln_b           (eps=1e-5)
  base  = silu(xn) @ base_weight.T             [B,S,O]
  t     = tanh(xn)
  basis = cos(pi*k*t), k=1..8
  spl   = einsum('bsig,oig->bso', basis, spline_weight)
  out   = base + spl

Strategy: data-parallel over batch (8 cores, one batch entry each, no
collectives).  Per core the whole thing is one K=18432 GEMM:
  out[o, t] = sum_k W_all[k, o] * panel[k, t]
where panel rows are [silu(xn); cos(1*pi*t); ...; cos(8*pi*t)] per
I-chunk, generated on-chip.  cos(k*pi*t) is built from
c1 = cos(pi*t) = 1 - 2*sin(pi*t/2)^2 via Chebyshev product
identities on the VectorEngine (ScalarE Sin is only valid on [-pi,pi]).
Weights are pre-transposed/pre-tiled on the host, cast to bf16; matmul
runs bf16 with f32 PSUM accumulation.

K-step order is ic-major: step s = ic*9 + m (m=0 silu, m=1..8 cos_m),
so the matmul consumes panel tiles in exactly the order generation
produces them.

Perf notes (measured on HW): the bf16 N=512 matmul stream floor is
~216 ns/MM and LDWEIGHTS hides fully as long as the weight DMAs stay
ahead.  To that end: output DMAs issue on the ACT HWDGE queue so the
SP queue only carries weight/x DMAs; weight DMAs move 8 k-steps per
transfer (kg=8) with a 5-deep pool; 5 o-tiles race the panel
generation (matching panel production rate ~8.7us/ic); the next
chunk's first two I-chunks are generated inside the tail of the
current mm sweep so the PE crosses chunk boundaries without idling;
200 warmup matmuls keep the HAM clock-gate open during the initial
LayerNorm; and a post-schedule pass (_optimize_sems) strips
unreferenced semaphore increments.
"""

import math
import sys
import types

try:  # some images lack antenv.axon_hooks, which bass_utils imports
    import antenv.axon_hooks  # noqa: F401
except Exception:
    try:
        import antenv
        _hooks = {}
        _m = types.ModuleType("antenv.axon_hooks")
        _m.set_axon_ntff_profile_hook = lambda h: _hooks.__setitem__("h", h)
        _m.get_axon_ntff_profile_hook = lambda: _hooks.get("h")
        sys.modules["antenv.axon_hooks"] = _m
        antenv.axon_hooks = _m
    except Exception:
        pass

import numpy as np
import ml_dtypes

import concourse.bass as bass
import concourse.mybir as mybir
import concourse.tile as tile
from concourse import bacc
from concourse import masks
from concourse.bass import ds, ts
from concourse.bass_utils import run_bass_kernel_spmd

F32 = mybir.dt.float32
BF16 = mybir.dt.bfloat16
AF = mybir.ActivationFunctionType
ALU = mybir.AluOpType

EPS = 1e-5

# geometry (full problem, per core)
B = 8
T = 2048          # tokens per core (= S, one batch entry per core)
I = 2048          # input dim
O = 2048          # output dim
G = 8             # cos harmonics
TCH = 512         # token chunk (matmul N)
NCH = T // TCH    # 4
NIC = I // 128    # 16 I-chunks
NM = G + 1        # 9 panel row-groups per ic (silu + 8 cos)
NK = NIC * NM     # 144 k-steps of 128
KG = 8            # k-steps per weight DMA group
NG = NK // KG     # 18
NOT = O // 128    # 16 o-tiles


def build_nc(nT=T, nI=I, nO=O, tch=TCH):
    nch = nT // tch
    nic = nI // 128
    nk = nic * NM
    n_ot = nO // 128
    ntt = tch // 128          # token-tiles per chunk
    kg = KG
    while nk % kg != 0:
        kg //= 2
    ng = nk // kg

    n_race = min(5, n_ot - 1) if n_ot > 1 else 1

    nc = bacc.Bacc("TRN2", target_bir_lowering=False, debug=False)
    x_ext = nc.declare_dram_parameter("x", [nT, nI], F32, isOutput=False)
    lnw_ext = nc.declare_dram_parameter("lnw", [nI], F32, isOutput=False)
    lnb_ext = nc.declare_dram_parameter("lnb", [nI], F32, isOutput=False)
    wt_ext = nc.declare_dram_parameter("wt", [n_ot, ng, 128, kg, 128], BF16, isOutput=False)
    out_ext = nc.declare_dram_parameter("out", [nO, nT], F32, isOutput=True)

    with tile.TileContext(nc) as tc:
        with (
            tc.tile_pool(name="consts", bufs=1) as consts,
            tc.tile_pool(name="xp", bufs=4) as xpool,
            tc.tile_pool(name="statp", bufs=2) as statp,
            tc.tile_pool(name="genp", bufs=1) as genp,
            tc.tile_pool(name="ladp", bufs=1) as ladp,
            tc.tile_pool(name="panelp", bufs=1) as panelp,
            tc.tile_pool(name="wp", bufs=5) as wp,
            tc.tile_pool(name="stgp", bufs=2) as stgp,
            tc.tile_pool(name="tpps", bufs=3, space="PSUM") as tpps,
            tc.tile_pool(name="mmps", bufs=5, space="PSUM") as mmps,
        ):
            identity = consts.tile([128, 128], F32)
            masks.make_identity(nc, identity[:])
            lnw_sb = consts.tile([128, nic], F32)
            nc.sync.dma_start(lnw_sb[:], lnw_ext.rearrange("(f p) -> p f", p=128))
            lnb_sb = consts.tile([128, nic], F32)
            nc.sync.dma_start(lnb_sb[:], lnb_ext.rearrange("(f p) -> p f", p=128))
            eps_sb = consts.tile([128, 1], F32)
            nc.vector.memset(eps_sb[:], EPS)
            zb = consts.tile([128, 128], BF16)
            nc.vector.memset(zb[:], 0.0)

            # PE warmup: keep HAM busy while the first chunk's LN runs so
            # the first real matmuls start at full clock.
            wps = mmps.tile([128, tch], F32, tag="ps", name="warm_ps")
            for _ in range(200):
                nc.tensor.matmul(wps[:, 0:128], zb[:], zb[:])

            state = {}
            tpm = {}

            def preamble(c):
                """x DMA + LN stats + in-place normalize for chunk c.
                Stats/normalize are per token-tile so the first tile is
                ready after one x DMA, not four."""
                xnts = []
                for j in range(ntt):
                    xt = xpool.tile([128, nI], F32, tag="xt")
                    nc.sync.dma_start(xt[:], x_ext[ds((c * ntt + j) * 128, 128), :])
                    bn6 = statp.tile([128, 4, 6], F32, tag="bn6")
                    for q in range(4):
                        nc.vector.bn_stats(
                            bn6[:, q, :], xt[:, ds(q * (nI // 4), nI // 4)]
                        )
                    stats = statp.tile([128, 2], F32, tag="stats")
                    nc.vector.bn_aggr(stats[:], bn6[:])
                    std = statp.tile([128, 1], F32, tag="std")
                    nc.scalar.activation(
                        std[:], stats[:, 1:2], AF.Sqrt, bias=eps_sb[:]
                    )
                    istd = statp.tile([128, 1], F32, tag="istd")
                    nc.vector.reciprocal(istd[:], std[:])
                    nmi = statp.tile([128, 1], F32, tag="nmi")
                    nc.vector.scalar_tensor_tensor(
                        nmi[:], stats[:, 0:1], -1.0, istd[:], ALU.mult, ALU.mult
                    )
                    # normalize in place: xn = (x - mu) * istd
                    nc.scalar.activation(
                        xt[:], xt[:], AF.Identity, bias=nmi[:], scale=istd[:],
                    )
                    xnts.append(xt)
                state[c] = xnts

            def transpose_ic(c, ic):
                """PE-transpose I-chunk ic of chunk c into a PSUM tile."""
                xnts = state[c]
                tp = tpps.tile([128, tch], F32, tag="tp", name=f"tp_{c}_{ic}")
                for j in range(ntt):
                    nc.tensor.transpose(
                        tp[:, ts(j, 128)], xnts[j][:, ts(ic, 128)], identity[:]
                    )
                tpm[(c, ic)] = tp
                return tp

            pre_ptiles = {}

            def gen_ic(c, ic, ptiles):
                """Transpose + tanh/silu + cheb ladder for I-chunk ic of
                chunk c, filling ptiles[9*ic : 9*(ic+1)]."""
                tp = tpm.pop((c, ic), None)
                if tp is None:
                    tp = transpose_ic(c, ic)
                lw = lnw_sb[:, ic : ic + 1]
                lb = lnb_sb[:, ic : ic + 1]

                def pt(m):
                    s = ic * NM + m
                    t_ = panelp.tile(
                        [128, tch], BF16, tag=f"p{s:03d}", name=f"panel_{c}_{s:03d}"
                    )
                    ptiles[s] = t_
                    return t_

                th = genp.tile([128, tch], F32, tag="th")
                nc.scalar.activation(th[:], tp[:], AF.Tanh, bias=lb, scale=lw)

                p0 = pt(0)
                nc.scalar.activation(p0[:], tp[:], AF.Silu, bias=lb, scale=lw)
                sh = genp.tile([128, tch], F32, tag="sh")
                nc.scalar.activation(sh[:], th[:], AF.Sin, scale=math.pi / 2)

                def lad(tag):
                    return ladp.tile(
                        [128, tch], F32, tag=tag, name=f"lad_{tag}_{c}_{ic}"
                    )

                def stt(out, a, s, b):
                    nc.vector.scalar_tensor_tensor(
                        out[:], a[:], s, b[:], ALU.mult, ALU.mult
                    )

                # c1 = 1 - 2*sh^2
                u = lad("u")
                stt(u, sh, -2.0, sh)
                c1 = lad("c1")
                nc.vector.tensor_scalar_add(c1[:], u[:], 1.0)
                # squares on ScalarE to offload DVE
                sq1 = lad("sq")
                nc.scalar.square(sq1[:], c1[:])
                c2 = lad("c2")
                nc.vector.tensor_scalar(c2[:], sq1[:], 2.0, -1.0, ALU.mult, ALU.add)
                # c3 = 2*c1*c2 - c1
                u3 = lad("u")
                stt(u3, c2, 2.0, c1)
                c3 = lad("c3")
                nc.vector.tensor_sub(c3[:], u3[:], c1[:])

                sq2 = lad("sq")
                nc.scalar.square(sq2[:], c2[:])
                c4 = lad("c4")
                nc.vector.tensor_scalar(c4[:], sq2[:], 2.0, -1.0, ALU.mult, ALU.add)
                # exports for m=1..4
                nc.scalar.copy(pt(1)[:], c1[:])
                nc.scalar.copy(pt(2)[:], c2[:])
                nc.scalar.copy(pt(3)[:], c3[:])
                nc.vector.tensor_copy(pt(4)[:], c4[:])
                # leaves m=5..8 straight to panel (bf16)
                u5 = lad("u")
                stt(u5, c3, 2.0, c2)
                p5 = pt(5)
                nc.vector.tensor_sub(p5[:], u5[:], c1[:])
                sq3 = lad("sq")
                nc.scalar.square(sq3[:], c3[:])
                nc.vector.tensor_scalar(
                    pt(6)[:], sq3[:], 2.0, -1.0, ALU.mult, ALU.add
                )
                u7 = lad("u")
                stt(u7, c4, 2.0, c3)
                nc.vector.tensor_sub(pt(7)[:], u7[:], c1[:])
                sq4 = lad("sq")
                nc.scalar.square(sq4[:], c4[:])
                p8 = pt(8)
                nc.vector.tensor_scalar(
                    p8[:], sq4[:], 2.0, -1.0, ALU.mult, ALU.add
                )

            def gen_chunk(c):
                """Panel gen for chunk c (skipping pre-generated I-chunks).
                o-tile 0..3's matmul groups are emitted interleaved so the
                TensorE does real GEMM work (and stays HAM-warm) while the
                panel is being generated."""
                ptiles, pre_ic = pre_ptiles.pop(c, ([None] * nk, 0))
                pss = [
                    mmps.tile([128, tch], F32, tag="ps", name=f"ps{r}_{c}")
                    for r in range(n_race)
                ]
                g_next = 0

                def race_mm(g_hi):
                    nonlocal g_next
                    for g in range(g_next, g_hi):
                        for r in range(n_race):
                            wg = wp.tile([128, kg, 128], BF16, tag="wg",
                                         name=f"wg{r}_{c}_{g}")
                            nc.sync.dma_start(wg[:], wt_ext[r, g])
                            for k8 in range(kg):
                                sidx = g * kg + k8
                                nc.tensor.matmul(
                                    pss[r][:], wg[:, k8, :], ptiles[sidx][:],
                                    start=(sidx == 0), stop=(sidx == nk - 1),
                                )
                    g_next = g_hi

                for ic in range(nic):
                    if ic >= pre_ic:
                        gen_ic(c, ic, ptiles)
                    race_mm((NM * (ic + 1)) // kg)
                race_mm(ng)
                for r in range(n_race):
                    stg = stgp.tile([128, tch], F32, tag="stg",
                                    name=f"stg{r}_{c}")
                    nc.vector.tensor_copy(stg[:], pss[r][:])
                    nc.scalar.dma_start(
                        out_ext[ds(r * 128, 128), ds(c * tch, tch)], stg[:]
                    )
                return ptiles

            def mm_chunk(c, ptiles, nxt=None):
                if nxt is not None:
                    nxt_ptiles = [None] * nk
                    pre_ptiles[nxt] = (nxt_ptiles, 4)
                for ot in range(n_race, n_ot):
                    ps = mmps.tile([128, tch], F32, tag="ps")
                    for g in range(ng):
                        if nxt is not None and ot == n_ot - 1:
                            if g == ng - 8:
                                gen_ic(nxt, 0, nxt_ptiles)
                            elif g == ng - 6:
                                gen_ic(nxt, 1, nxt_ptiles)
                            elif g == ng - 4:
                                gen_ic(nxt, 2, nxt_ptiles)
                            elif g == ng - 2:
                                gen_ic(nxt, 3, nxt_ptiles)
                        wg = wp.tile([128, kg, 128], BF16, tag="wg")
                        nc.sync.dma_start(wg[:], wt_ext[ot, g])
                        for k8 in range(kg):
                            s = g * kg + k8
                            nc.tensor.matmul(
                                ps[:],
                                wg[:, k8, :],
                                ptiles[s][:],
                                start=(s == 0),
                                stop=(s == nk - 1),
                            )
                    stg = stgp.tile([128, tch], F32, tag="stg")
                    nc.vector.tensor_copy(stg[:], ps[:])
                    nc.scalar.dma_start(
                        out_ext[ds(ot * 128, 128), ds(c * tch, tch)], stg[:]
                    )

            preamble(0)
            for c in range(nch):
                ptiles = gen_chunk(c)
                if c + 1 < nch:
                    preamble(c + 1)
                mm_chunk(c, ptiles, nxt=(c + 1) if c + 1 < nch else None)

    _optimize_sems(nc)
    nc.compile()
    return nc


def _optimize_sems(nc):
    """Post-schedule IR pass: engine instructions complete in queue order, so
    a monotone per-engine counter semaphore only needs an increment at the
    positions some wait actually references.  Strip the rest and renumber the
    wait thresholds.  Also drop waits dominated by an earlier wait on the
    same engine queue.  Semaphores touched by DMA completions or any
    non-inc update are left alone."""
    ENG_FIFO = {
        mybir.EngineType.PE,
        mybir.EngineType.Activation,
        mybir.EngineType.DVE,
        mybir.EngineType.Pool,
        mybir.EngineType.SP,
    }
    f = nc.m.functions[0]
    insts = [i for bb in f.blocks for i in bb.instructions]

    upd_insts = {}   # sem id -> list of (inst, engine, value) in program order
    upd_ok = {}      # sem id -> eligible for stripping
    waited = {}      # sem id -> set of imm values referenced
    wait_bad = set()  # sems with register/non-ge waits
    for inst in insts:
        si = inst.sync_info
        if not si:
            continue
        is_dma = "DMA" in type(inst).__name__ or "Dma" in type(inst).__name__
        for u in (si.on_update or []):
            upd_insts.setdefault(u.id, []).append((inst, u))
            ok = upd_ok.get(u.id, True)
            if (is_dma or inst.engine not in ENG_FIFO
                    or u.update_mode != "sem-inc" or u.update_value != 1
                    or u.update_reg is not None):
                ok = False
            if any(e != inst.engine for (pi, pu) in upd_insts[u.id] for e in [pi.engine]):
                ok = False
            upd_ok[u.id] = ok
        for w in (si.on_wait or []):
            if w.wait_reg is not None or w.wait_mode != "sem-ge-imm":
                wait_bad.add(w.id)
            else:
                waited.setdefault(w.id, set()).add(w.wait_value)

    # monotone sems: every update is a positive immediate inc/add (wait-ge on
    # these can never be un-satisfied, so dominated waits are droppable)
    monotone = set()
    for sid, lst in upd_insts.items():
        if all(u.update_mode in ("sem-inc", "sem-add-imm")
               and u.update_reg is None and (u.update_value or 0) > 0
               for (_, u) in lst):
            monotone.add(sid)

    remap = {}  # sem id -> {old_val: new_val}
    keep_pos = {}  # sem id -> set of cumulative counts to keep
    for sid, lst in upd_insts.items():
        if not upd_ok.get(sid) or sid in wait_bad:
            continue
        total = len(lst)
        refs = sorted(v for v in waited.get(sid, ()) if 1 <= v <= total)
        if any(v > total or v < 1 for v in waited.get(sid, ())):
            continue
        if total not in refs:
            refs.append(total)  # keep the final count reachable for drains
        remap[sid] = {v: i + 1 for i, v in enumerate(refs)}
        keep_pos[sid] = set(refs)

    n_strip = n_keep = n_wdrop = 0
    counts = {sid: 0 for sid in remap}
    eng_wait_max = {}  # (engine, sem) -> max value already waited on that queue
    for inst in insts:
        si = inst.sync_info
        if not si:
            continue
        new_upd, new_wait, changed = [], [], False
        for u in (si.on_update or []):
            if u.id in remap:
                counts[u.id] += 1
                if counts[u.id] in keep_pos[u.id]:
                    new_upd.append(u)
                    n_keep += 1
                else:
                    changed = True
                    n_strip += 1
            else:
                new_upd.append(u)
        for w in (si.on_wait or []):
            v = w.wait_value
            if w.id in remap and w.wait_reg is None and w.wait_mode == "sem-ge-imm":
                v = remap[w.id][w.wait_value]
            key = (inst.engine, w.id)
            is_imm = w.wait_reg is None and w.wait_mode == "sem-ge-imm"
            if is_imm and w.id in monotone and eng_wait_max.get(key, 0) >= v:
                changed = True
                n_wdrop += 1
                continue
            if is_imm and w.id in monotone:
                eng_wait_max[key] = max(eng_wait_max.get(key, 0), v)
            if v != w.wait_value:
                w = mybir.SyncWait(sync_type=w.sync_type, id=w.id,
                                   ant_name=w.ant_name, wait_mode=w.wait_mode,
                                   wait_value=v, wait_reg=None)
                changed = True
            new_wait.append(w)
        if changed:
            inst.sync_info = mybir.SyncInfo(on_wait=new_wait, on_update=new_upd)
    print(f"_optimize_sems: stripped {n_strip} incs (kept {n_keep}), "
          f"dropped {n_wdrop} dominated waits")


def prep_weights(base_weight, spline_weight, nO=O, nI=I):
    """Host-side: build bf16 W_all in ic-major k-step order, pre-tiled
    for contiguous [128, kg, 128] DMAs: wt[ot, grp, k_in, ks, o_in]."""
    nic = nI // 128
    nk = nic * NM
    n_ot = nO // 128
    kg = KG
    while nk % kg != 0:
        kg //= 2
    ng = nk // kg
    w = np.empty((NM, nI, nO), np.float32)
    w[0] = base_weight.T                      # [i, o]
    for g in range(G):
        w[1 + g] = spline_weight[:, :, g].T   # [i, o]
    # m-major [9, nic, 128, nO] -> ic-major [nic, 9, 128, nO] -> [nk*128, nO]
    w = w.reshape(NM, nic, 128, nO).transpose(1, 0, 2, 3).reshape(nk * 128, nO)
    w = w.reshape(ng, kg, 128, n_ot, 128).transpose(3, 0, 2, 1, 4)
    return np.ascontiguousarray(w.astype(ml_dtypes.bfloat16))


_NC_CACHE = {}


def _get_nc():
    if "nc" not in _NC_CACHE:
        _NC_CACHE["nc"] = build_nc()
    return _NC_CACHE["nc"]


def kernel(x, ln_weight, ln_bias, base_weight, spline_weight):
    x = np.asarray(x, np.float32)
    ln_weight = np.asarray(ln_weight, np.float32)
    ln_bias = np.asarray(ln_bias, np.float32)
    wt = prep_weights(np.asarray(base_weight, np.float32),
                      np.asarray(spline_weight, np.float32))
    nc = _get_nc()
    in_maps = [
        {
            "x": np.ascontiguousarray(x[b]),
            "lnw": ln_weight,
            "lnb": ln_bias,
            "wt": wt,
        }
        for b in range(B)
    ]
    res = run_bass_kernel_spmd(nc, in_maps, core_ids=list(range(B)))
    out = np.stack([res.results[b]["out"].T for b in range(B)])
    return np.ascontiguousarray(out.astype(np.float32))



# revision 22
# speedup vs baseline: 1.1433x; 1.0017x over previous
"""Trainium2 Bass kernel for AdvancedKANLayer.

Math (per reference):
  xn    = LayerNorm(x) * ln_w + ln_b           (eps=1e-5)
  base  = silu(xn) @ base_weight.T             [B,S,O]
  t     = tanh(xn)
  basis = cos(pi*k*t), k=1..8
  spl   = einsum('bsig,oig->bso', basis, spline_weight)
  out   = base + spl

Strategy: data-parallel over batch (8 cores, one batch entry each, no
collectives).  Per core the whole thing is one K=18432 GEMM:
  out[o, t] = sum_k W_all[k, o] * panel[k, t]
where panel rows are [silu(xn); cos(1*pi*t); ...; cos(8*pi*t)] per
I-chunk, generated on-chip.  cos(k*pi*t) is built from
c1 = cos(pi*t) = 1 - 2*sin(pi*t/2)^2 via Chebyshev product
identities on the VectorEngine (ScalarE Sin is only valid on [-pi,pi]).
Weights are pre-transposed/pre-tiled on the host, cast to bf16; matmul
runs bf16 with f32 PSUM accumulation.

K-step order is ic-major: step s = ic*9 + m (m=0 silu, m=1..8 cos_m),
so the matmul consumes panel tiles in exactly the order generation
produces them.

Perf notes (measured on HW): the bf16 N=512 matmul stream floor is
~216 ns/MM and LDWEIGHTS hides fully as long as the weight DMAs stay
ahead.  To that end: output DMAs issue on the ACT HWDGE queue so the
SP queue only carries weight/x DMAs; weight DMAs move 8 k-steps per
transfer (kg=8) with a 5-deep pool; 5 o-tiles race the panel
generation (matching panel production rate ~8.7us/ic); the next
chunk's first two I-chunks are generated inside the tail of the
current mm sweep so the PE crosses chunk boundaries without idling;
200 warmup matmuls keep the HAM clock-gate open during the initial
LayerNorm; and a post-schedule pass (_optimize_sems) strips
unreferenced semaphore increments.
"""

import math
import sys
import types

try:  # some images lack antenv.axon_hooks, which bass_utils imports
    import antenv.axon_hooks  # noqa: F401
except Exception:
    try:
        import antenv
        _hooks = {}
        _m = types.ModuleType("antenv.axon_hooks")
        _m.set_axon_ntff_profile_hook = lambda h: _hooks.__setitem__("h", h)
        _m.get_axon_ntff_profile_hook = lambda: _hooks.get("h")
        sys.modules["antenv.axon_hooks"] = _m
        antenv.axon_hooks = _m
    except Exception:
        pass

import numpy as np
import ml_dtypes

import concourse.bass as bass
import concourse.mybir as mybir
import concourse.tile as tile
from concourse import bacc
from concourse import masks
from concourse.bass import ds, ts
from concourse.bass_utils import run_bass_kernel_spmd

F32 = mybir.dt.float32
BF16 = mybir.dt.bfloat16
AF = mybir.ActivationFunctionType
ALU = mybir.AluOpType

EPS = 1e-5

# geometry (full problem, per core)
B = 8
T = 2048          # tokens per core (= S, one batch entry per core)
I = 2048          # input dim
O = 2048          # output dim
G = 8             # cos harmonics
TCH = 512         # token chunk (matmul N)
NCH = T // TCH    # 4
NIC = I // 128    # 16 I-chunks
NM = G + 1        # 9 panel row-groups per ic (silu + 8 cos)
NK = NIC * NM     # 144 k-steps of 128
KG = 8            # k-steps per weight DMA group
NG = NK // KG     # 18
NOT = O // 128    # 16 o-tiles


def build_nc(nT=T, nI=I, nO=O, tch=TCH):
    nch = nT // tch
    nic = nI // 128
    nk = nic * NM
    n_ot = nO // 128
    ntt = tch // 128          # token-tiles per chunk
    kg = KG
    while nk % kg != 0:
        kg //= 2
    ng = nk // kg

    n_race = min(5, n_ot - 1) if n_ot > 1 else 1

    nc = bacc.Bacc("TRN2", target_bir_lowering=False, debug=False)
    x_ext = nc.declare_dram_parameter("x", [nT, nI], F32, isOutput=False)
    lnw_ext = nc.declare_dram_parameter("lnw", [nI], F32, isOutput=False)
    lnb_ext = nc.declare_dram_parameter("lnb", [nI], F32, isOutput=False)
    wt_ext = nc.declare_dram_parameter("wt", [n_ot, ng, 128, kg, 128], BF16, isOutput=False)
    out_ext = nc.declare_dram_parameter("out", [nO, nT], F32, isOutput=True)

    with tile.TileContext(nc) as tc:
        with (
            tc.tile_pool(name="consts", bufs=1) as consts,
            tc.tile_pool(name="xp", bufs=4) as xpool,
            tc.tile_pool(name="statp", bufs=2) as statp,
            tc.tile_pool(name="genp", bufs=1) as genp,
            tc.tile_pool(name="ladp", bufs=1) as ladp,
            tc.tile_pool(name="panelp", bufs=1) as panelp,
            tc.tile_pool(name="wp", bufs=5) as wp,
            tc.tile_pool(name="stgp", bufs=2) as stgp,
            tc.tile_pool(name="tpps", bufs=3, space="PSUM") as tpps,
            tc.tile_pool(name="mmps", bufs=5, space="PSUM") as mmps,
        ):
            identity = consts.tile([128, 128], F32)
            masks.make_identity(nc, identity[:])
            lnw_sb = consts.tile([128, nic], F32)
            nc.sync.dma_start(lnw_sb[:], lnw_ext.rearrange("(f p) -> p f", p=128))
            lnb_sb = consts.tile([128, nic], F32)
            nc.sync.dma_start(lnb_sb[:], lnb_ext.rearrange("(f p) -> p f", p=128))
            eps_sb = consts.tile([128, 1], F32)
            nc.vector.memset(eps_sb[:], EPS)
            zb = consts.tile([128, 128], BF16)
            nc.vector.memset(zb[:], 0.0)

            # PE warmup: keep HAM busy while the first chunk's LN runs so
            # the first real matmuls start at full clock.
            wps = mmps.tile([128, tch], F32, tag="ps", name="warm_ps")
            for _ in range(200):
                nc.tensor.matmul(wps[:, 0:128], zb[:], zb[:])

            state = {}
            tpm = {}

            def preamble(c):
                """x DMA + LN stats + in-place normalize for chunk c.
                Stats/normalize are per token-tile so the first tile is
                ready after one x DMA, not four."""
                xnts = []
                for j in range(ntt):
                    xt = xpool.tile([128, nI], F32, tag="xt")
                    nc.sync.dma_start(xt[:], x_ext[ds((c * ntt + j) * 128, 128), :])
                    bn6 = statp.tile([128, 4, 6], F32, tag="bn6")
                    for q in range(4):
                        nc.vector.bn_stats(
                            bn6[:, q, :], xt[:, ds(q * (nI // 4), nI // 4)]
                        )
                    stats = statp.tile([128, 2], F32, tag="stats")
                    nc.vector.bn_aggr(stats[:], bn6[:])
                    std = statp.tile([128, 1], F32, tag="std")
                    nc.scalar.activation(
                        std[:], stats[:, 1:2], AF.Sqrt, bias=eps_sb[:]
                    )
                    istd = statp.tile([128, 1], F32, tag="istd")
                    nc.vector.reciprocal(istd[:], std[:])
                    nmi = statp.tile([128, 1], F32, tag="nmi")
                    nc.vector.scalar_tensor_tensor(
                        nmi[:], stats[:, 0:1], -1.0, istd[:], ALU.mult, ALU.mult
                    )
                    # normalize in place: xn = (x - mu) * istd
                    nc.scalar.activation(
                        xt[:], xt[:], AF.Identity, bias=nmi[:], scale=istd[:],
                    )
                    xnts.append(xt)
                state[c] = xnts

            def transpose_ic(c, ic):
                """PE-transpose I-chunk ic of chunk c into a PSUM tile."""
                xnts = state[c]
                tp = tpps.tile([128, tch], F32, tag="tp", name=f"tp_{c}_{ic}")
                for j in range(ntt):
                    nc.tensor.transpose(
                        tp[:, ts(j, 128)], xnts[j][:, ts(ic, 128)], identity[:]
                    )
                tpm[(c, ic)] = tp
                return tp

            pre_ptiles = {}

            def gen_ic(c, ic, ptiles):
                """Transpose + tanh/silu + cheb ladder for I-chunk ic of
                chunk c, filling ptiles[9*ic : 9*(ic+1)]."""
                tp = tpm.pop((c, ic), None)
                if tp is None:
                    tp = transpose_ic(c, ic)
                lw = lnw_sb[:, ic : ic + 1]
                lb = lnb_sb[:, ic : ic + 1]

                def pt(m):
                    s = ic * NM + m
                    t_ = panelp.tile(
                        [128, tch], BF16, tag=f"p{s:03d}", name=f"panel_{c}_{s:03d}"
                    )
                    ptiles[s] = t_
                    return t_

                th = genp.tile([128, tch], F32, tag="th")
                nc.scalar.activation(th[:], tp[:], AF.Tanh, bias=lb, scale=lw)

                p0 = pt(0)
                nc.scalar.activation(p0[:], tp[:], AF.Silu, bias=lb, scale=lw)
                sh = genp.tile([128, tch], F32, tag="sh")
                nc.scalar.activation(sh[:], th[:], AF.Sin, scale=math.pi / 2)

                def lad(tag):
                    return ladp.tile(
                        [128, tch], F32, tag=tag, name=f"lad_{tag}_{c}_{ic}"
                    )

                def stt(out, a, s, b):
                    nc.vector.scalar_tensor_tensor(
                        out[:], a[:], s, b[:], ALU.mult, ALU.mult
                    )

                # c1 = 1 - 2*sh^2
                u = lad("u")
                stt(u, sh, -2.0, sh)
                c1 = lad("c1")
                nc.vector.tensor_scalar_add(c1[:], u[:], 1.0)
                # squares on ScalarE to offload DVE
                sq1 = lad("sq")
                nc.scalar.square(sq1[:], c1[:])
                c2 = lad("c2")
                nc.vector.tensor_scalar(c2[:], sq1[:], 2.0, -1.0, ALU.mult, ALU.add)
                # c3 = 2*c1*c2 - c1
                u3 = lad("u")
                stt(u3, c2, 2.0, c1)
                c3 = lad("c3")
                nc.vector.tensor_sub(c3[:], u3[:], c1[:])

                sq2 = lad("sq")
                nc.scalar.square(sq2[:], c2[:])
                c4 = lad("c4")
                nc.vector.tensor_scalar(c4[:], sq2[:], 2.0, -1.0, ALU.mult, ALU.add)
                # exports for m=1..4
                nc.scalar.copy(pt(1)[:], c1[:])
                nc.scalar.copy(pt(2)[:], c2[:])
                nc.scalar.copy(pt(3)[:], c3[:])
                nc.vector.tensor_copy(pt(4)[:], c4[:])
                # leaves m=5..8 straight to panel (bf16)
                u5 = lad("u")
                stt(u5, c3, 2.0, c2)
                p5 = pt(5)
                nc.vector.tensor_sub(p5[:], u5[:], c1[:])
                sq3 = lad("sq")
                nc.scalar.square(sq3[:], c3[:])
                nc.vector.tensor_scalar(
                    pt(6)[:], sq3[:], 2.0, -1.0, ALU.mult, ALU.add
                )
                u7 = lad("u")
                stt(u7, c4, 2.0, c3)
                nc.vector.tensor_sub(pt(7)[:], u7[:], c1[:])
                sq4 = lad("sq")
                nc.scalar.square(sq4[:], c4[:])
                p8 = pt(8)
                nc.vector.tensor_scalar(
                    p8[:], sq4[:], 2.0, -1.0, ALU.mult, ALU.add
                )

            def gen_chunk(c):
                """Panel gen for chunk c (skipping pre-generated I-chunks).
                o-tile 0..3's matmul groups are emitted interleaved so the
                TensorE does real GEMM work (and stays HAM-warm) while the
                panel is being generated."""
                ptiles, pre_ic = pre_ptiles.pop(c, ([None] * nk, 0))
                pss = [
                    mmps.tile([128, tch], F32, tag="ps", name=f"ps{r}_{c}")
                    for r in range(n_race)
                ]
                g_next = 0

                def race_mm(g_hi):
                    nonlocal g_next
                    for g in range(g_next, g_hi):
                        for r in range(n_race):
                            wg = wp.tile([128, kg, 128], BF16, tag="wg",
                                         name=f"wg{r}_{c}_{g}")
                            nc.sync.dma_start(wg[:], wt_ext[r, g])
                            for k8 in range(kg):
                                sidx = g * kg + k8
                                nc.tensor.matmul(
                                    pss[r][:], wg[:, k8, :], ptiles[sidx][:],
                                    start=(sidx == 0), stop=(sidx == nk - 1),
                                )
                    g_next = g_hi

                for ic in range(nic):
                    if ic >= pre_ic:
                        gen_ic(c, ic, ptiles)
                    race_mm((NM * (ic + 1)) // kg)
                race_mm(ng)
                for r in range(n_race):
                    stg = stgp.tile([128, tch], F32, tag="stg",
                                    name=f"stg{r}_{c}")
                    nc.vector.tensor_copy(stg[:], pss[r][:])
                    nc.scalar.dma_start(
                        out_ext[ds(r * 128, 128), ds(c * tch, tch)], stg[:]
                    )
                return ptiles

            def mm_chunk(c, ptiles, nxt=None):
                if nxt is not None:
                    nxt_ptiles = [None] * nk
                    pre_ptiles[nxt] = (nxt_ptiles, 6)
                for ot in range(n_race, n_ot):
                    ps = mmps.tile([128, tch], F32, tag="ps")
                    for g in range(ng):
                        if nxt is not None and ot == n_ot - 1:
                            if g >= ng - 12 and g % 2 == 0:
                                ici = (g - (ng - 12)) // 2
                                gen_ic(nxt, ici, nxt_ptiles)
                        wg = wp.tile([128, kg, 128], BF16, tag="wg")
                        nc.sync.dma_start(wg[:], wt_ext[ot, g])
                        for k8 in range(kg):
                            s = g * kg + k8
                            nc.tensor.matmul(
                                ps[:],
                                wg[:, k8, :],
                                ptiles[s][:],
                                start=(s == 0),
                                stop=(s == nk - 1),
                            )
                    stg = stgp.tile([128, tch], F32, tag="stg")
                    nc.vector.tensor_copy(stg[:], ps[:])
                    nc.scalar.dma_start(
                        out_ext[ds(ot * 128, 128), ds(c * tch, tch)], stg[:]
                    )

            preamble(0)
            for c in range(nch):
                ptiles = gen_chunk(c)
                if c + 1 < nch:
                    preamble(c + 1)
                mm_chunk(c, ptiles, nxt=(c + 1) if c + 1 < nch else None)

    _optimize_sems(nc)
    nc.compile()
    return nc


def _optimize_sems(nc):
    """Post-schedule IR pass: engine instructions complete in queue order, so
    a monotone per-engine counter semaphore only needs an increment at the
    positions some wait actually references.  Strip the rest and renumber the
    wait thresholds.  Also drop waits dominated by an earlier wait on the
    same engine queue.  Semaphores touched by DMA completions or any
    non-inc update are left alone."""
    ENG_FIFO = {
        mybir.EngineType.PE,
        mybir.EngineType.Activation,
        mybir.EngineType.DVE,
        mybir.EngineType.Pool,
        mybir.EngineType.SP,
    }
    f = nc.m.functions[0]
    insts = [i for bb in f.blocks for i in bb.instructions]

    upd_insts = {}   # sem id -> list of (inst, engine, value) in program order
    upd_ok = {}      # sem id -> eligible for stripping
    waited = {}      # sem id -> set of imm values referenced
    wait_bad = set()  # sems with register/non-ge waits
    for inst in insts:
        si = inst.sync_info
        if not si:
            continue
        is_dma = "DMA" in type(inst).__name__ or "Dma" in type(inst).__name__
        for u in (si.on_update or []):
            upd_insts.setdefault(u.id, []).append((inst, u))
            ok = upd_ok.get(u.id, True)
            if (is_dma or inst.engine not in ENG_FIFO
                    or u.update_mode != "sem-inc" or u.update_value != 1
                    or u.update_reg is not None):
                ok = False
            if any(e != inst.engine for (pi, pu) in upd_insts[u.id] for e in [pi.engine]):
                ok = False
            upd_ok[u.id] = ok
        for w in (si.on_wait or []):
            if w.wait_reg is not None or w.wait_mode != "sem-ge-imm":
                wait_bad.add(w.id)
            else:
                waited.setdefault(w.id, set()).add(w.wait_value)

    # monotone sems: every update is a positive immediate inc/add (wait-ge on
    # these can never be un-satisfied, so dominated waits are droppable)
    monotone = set()
    for sid, lst in upd_insts.items():
        if all(u.update_mode in ("sem-inc", "sem-add-imm")
               and u.update_reg is None and (u.update_value or 0) > 0
               for (_, u) in lst):
            monotone.add(sid)

    remap = {}  # sem id -> {old_val: new_val}
    keep_pos = {}  # sem id -> set of cumulative counts to keep
    for sid, lst in upd_insts.items():
        if not upd_ok.get(sid) or sid in wait_bad:
            continue
        total = len(lst)
        refs = sorted(v for v in waited.get(sid, ()) if 1 <= v <= total)
        if any(v > total or v < 1 for v in waited.get(sid, ())):
            continue
        if total not in refs:
            refs.append(total)  # keep the final count reachable for drains
        remap[sid] = {v: i + 1 for i, v in enumerate(refs)}
        keep_pos[sid] = set(refs)

    n_strip = n_keep = n_wdrop = 0
    counts = {sid: 0 for sid in remap}
    eng_wait_max = {}  # (engine, sem) -> max value already waited on that queue
    for inst in insts:
        si = inst.sync_info
        if not si:
            continue
        new_upd, new_wait, changed = [], [], False
        for u in (si.on_update or []):
            if u.id in remap:
                counts[u.id] += 1
                if counts[u.id] in keep_pos[u.id]:
                    new_upd.append(u)
                    n_keep += 1
                else:
                    changed = True
                    n_strip += 1
            else:
                new_upd.append(u)
        for w in (si.on_wait or []):
            v = w.wait_value
            if w.id in remap and w.wait_reg is None and w.wait_mode == "sem-ge-imm":
                v = remap[w.id][w.wait_value]
            key = (inst.engine, w.id)
            is_imm = w.wait_reg is None and w.wait_mode == "sem-ge-imm"
            if is_imm and w.id in monotone and eng_wait_max.get(key, 0) >= v:
                changed = True
                n_wdrop += 1
                continue
            if is_imm and w.id in monotone:
                eng_wait_max[key] = max(eng_wait_max.get(key, 0), v)
            if v != w.wait_value:
                w = mybir.SyncWait(sync_type=w.sync_type, id=w.id,
                                   ant_name=w.ant_name, wait_mode=w.wait_mode,
                                   wait_value=v, wait_reg=None)
                changed = True
            new_wait.append(w)
        if changed:
            inst.sync_info = mybir.SyncInfo(on_wait=new_wait, on_update=new_upd)
    print(f"_optimize_sems: stripped {n_strip} incs (kept {n_keep}), "
          f"dropped {n_wdrop} dominated waits")


def prep_weights(base_weight, spline_weight, nO=O, nI=I):
    """Host-side: build bf16 W_all in ic-major k-step order, pre-tiled
    for contiguous [128, kg, 128] DMAs: wt[ot, grp, k_in, ks, o_in]."""
    nic = nI // 128
    nk = nic * NM
    n_ot = nO // 128
    kg = KG
    while nk % kg != 0:
        kg //= 2
    ng = nk // kg
    w = np.empty((NM, nI, nO), np.float32)
    w[0] = base_weight.T                      # [i, o]
    for g in range(G):
        w[1 + g] = spline_weight[:, :, g].T   # [i, o]
    # m-major [9, nic, 128, nO] -> ic-major [nic, 9, 128, nO] -> [nk*128, nO]
    w = w.reshape(NM, nic, 128, nO).transpose(1, 0, 2, 3).reshape(nk * 128, nO)
    w = w.reshape(ng, kg, 128, n_ot, 128).transpose(3, 0, 2, 1, 4)
    return np.ascontiguousarray(w.astype(ml_dtypes.bfloat16))


_NC_CACHE = {}


def _get_nc():
    if "nc" not in _NC_CACHE:
        _NC_CACHE["nc"] = build_nc()
    return _NC_CACHE["nc"]


def kernel(x, ln_weight, ln_bias, base_weight, spline_weight):
    x = np.asarray(x, np.float32)
    ln_weight = np.asarray(ln_weight, np.float32)
    ln_bias = np.asarray(ln_bias, np.float32)
    wt = prep_weights(np.asarray(base_weight, np.float32),
                      np.asarray(spline_weight, np.float32))
    nc = _get_nc()
    in_maps = [
        {
            "x": np.ascontiguousarray(x[b]),
            "lnw": ln_weight,
            "lnb": ln_bias,
            "wt": wt,
        }
        for b in range(B)
    ]
    res = run_bass_kernel_spmd(nc, in_maps, core_ids=list(range(B)))
    out = np.stack([res.results[b]["out"].T for b in range(B)])
    return np.ascontiguousarray(out.astype(np.float32))

